# revision 1
# baseline (speedup 1.0000x reference)
"""AttentiveFP forward pass as a Bass/Tile kernel on 8 Trainium2 NeuronCores.

Strategy: data-parallel by graph blocks (256 graphs/core); edges assigned to
the core owning their dst node (edges freely cross cores); per-core windowed
segment-softmax aggregation via selection-matrix matmuls on the PE; node
features kept transposed on-chip so GATv2+GRU elementwise runs with
per-feature biases as per-partition ACT biases; node feature tables exchanged
between layers with AllGather collectives; per-edge source rows fetched with
indirect DMA gathers.
"""
import sys, os
sys.path.insert(0, '/opt/trn_rl_repo')
import numpy as np
from contextlib import ExitStack

import concourse.bass as bass
import concourse.mybir as mybir
import concourse.tile as tile
from concourse.bass import IndirectOffsetOnAxis
from concourse.mybir import AluOpType as alu, ActivationFunctionType as act

G_DEFAULT = 2048


def preprocess(edge_index, batch, n_cores=8, G=2048, CW=5):
    src = np.asarray(edge_index[0]).astype(np.int64)
    dst = np.asarray(edge_index[1]).astype(np.int64)
    batch = np.asarray(batch).astype(np.int64)
    N = batch.shape[0]
    GPC = G // n_cores
    gstart = np.searchsorted(batch, np.arange(0, G + 1, GPC))
    ncounts = np.diff(gstart)
    NLOC = int(np.ceil(ncounts.max() / 128) * 128)
    NWIN = NLOC // 128
    NCH = NWIN * CW

    node_owner = np.searchsorted(gstart, np.arange(N), side='right') - 1
    ag_row = (node_owner * NLOC + (np.arange(N) - gstart[node_owner])).astype(np.int64)
    owner = node_owner[dst]

    cores = []
    for c in range(n_cores):
        ns, ne = int(gstart[c]), int(gstart[c + 1])
        nn = ne - ns
        m = owner == c
        eidx = np.nonzero(m)[0]
        dl = dst[eidx] - ns
        order = np.argsort(dl, kind='stable')
        eidx = eidx[order]; dl = dl[order]
        win = dl // 128
        counts = np.bincount(win, minlength=NWIN)
        assert counts.max() <= CW * 128, f"window overflow {counts.max()}"
        pos = np.concatenate([[0], np.cumsum(counts)])[:-1]
        within = np.arange(len(dl)) - pos[win]
        slots = (win * CW * 128 + within).astype(np.int64)

        sl_src_ag = np.zeros(NCH * 128, np.int32)          # gather row in ag table
        sl_dstloc = np.full(NCH * 128, -1.0, np.float32)   # dst within window, -1 pad
        sl_edge = np.zeros(NCH * 128, np.int64)            # original edge id
        sl_fill = np.zeros(NCH * 128, bool)
        sl_src_ag[slots] = ag_row[src[eidx]]
        sl_dstloc[slots] = (dl % 128).astype(np.float32)
        sl_edge[slots] = eidx
        sl_fill[slots] = True

        # device-layout meta, per window loads:
        # srcblk [NWIN, 128, CW] int32 ; dstlocblk [NWIN, 128, CW] f32 ;
        # dstrowblk [NWIN, 1, CW*128] f32
        srcblk = sl_src_ag.reshape(NWIN, CW, 128).transpose(0, 2, 1).copy()
        dstlocblk = sl_dstloc.reshape(NWIN, CW, 128).transpose(0, 2, 1).copy()
        dstrowblk = sl_dstloc.reshape(NWIN, 1, CW * 128).copy()

        # mol phase: node chunk k -> graphs gloc (local graph id 0..GPC-1), pad -1
        gloc = np.full(NLOC, -1.0, np.float32)
        gloc[:nn] = (batch[ns:ne] - c * GPC).astype(np.float32)
        glocblk = gloc.reshape(NWIN, 128, 1).copy()

        cores.append(dict(ns=ns, ne=ne, nn=nn,
                          srcblk=srcblk, dstlocblk=dstlocblk, dstrowblk=dstrowblk,
                          glocblk=glocblk, sl_edge=sl_edge, sl_fill=sl_fill))
    return dict(cores=cores, gstart=gstart, NLOC=NLOC, NWIN=NWIN, NCH=NCH, CW=CW,
                GPC=GPC, n_cores=n_cores)

# ---------------- walrus sync-wait splitting ----------------
MAX_WAITS = 1

def split_waits(nc):
    eng_map = nc.engines
    for bbname, bassbb in nc.bb_map.items():
        insts = bassbb.bb.instructions
        i = 0
        while i < len(insts):
            inst = insts[i]
            si = inst.sync_info
            if si is not None and si.on_wait is not None and len(si.on_wait) > MAX_WAITS:
                waits = list(si.on_wait)
                si.on_wait = waits[-MAX_WAITS:]
                rest = waits[:-MAX_WAITS]
                for j in range(0, len(rest), MAX_WAITS):
                    eng = eng_map[inst.engine]
                    nop = eng.nop(nofuse=True)
                    nop_inst = nop.ins
                    for obb in nc.bb_map.values():
                        lst = obb.bb.instructions
                        for k in range(len(lst) - 1, -1, -1):
                            if lst[k].name == nop_inst.name:
                                del lst[k]
                                break
                    nsi = nop_inst.sync_info
                    chunk = rest[j:j + MAX_WAITS]
                    if nsi is None:
                        nop_inst.sync_info = mybir.SyncInfo(on_wait=chunk, on_update=[])
                    else:
                        nsi.on_wait = chunk
                    insts.insert(i, nop_inst)
                    i += 1
            i += 1


class TileContextFixed(tile.TileContext):
    def __exit__(self, *args):
        r = super().__exit__(*args)
        split_waits(self.nc)
        return r


F32 = mybir.dt.float32

F32 = mybir.dt.float32
I32 = mybir.dt.int32
EPS = 1e-30


def wpack_layout():
    """Returns (layout dict name->(off, cols), total_cols). All blocks [128, cols]."""
    L = {}
    off = 0
    def add(name, cols):
        nonlocal off
        L[name] = (off, cols)
        off += cols
    add("iota_sq", 128)
    add("iota256", 256)
    add("iota_col", 1)
    add("ones_col", 1)
    add("attl_sq", 256)      # g_att_l replicated rows
    add("attm_sq", 256)      # mol_att replicated rows
    for l in range(3):
        add(f"att{l}_sq", 256)
    add("W1T", 2 * 256)      # u = x @ W1.T : rhs chunks [128,256] x2
    for i in range(4):       # 0..2 atom, 3 mol
        add(f"WlTr{i}", 2 * 256)
        add(f"WrTr{i}", 2 * 256)
        for k in range(2):
            for b in range(2):
                add(f"WrTl{i}_{k}{b}", 128)
    for k in range(2):
        for b in range(2):
            add(f"gl2T_{k}{b}", 128)
    add("gb2", 2)            # g_bias2 cols x2 blocks
    add("attr_col", 2)       # g_att_r as 2 col chunks [128,1]
    for g in range(5):       # gru0, agru0..2, mgru
        for j in range(12):
            for b in range(2):
                add(f"gru{g}_w{j}{b}", 128)
        for j in range(4):   # br, bz, bin, bhn
            for b in range(2):
                add(f"gru{g}_b{j}{b}", 1)
    for i in range(3):
        add(f"ab{i}", 2)     # atom bias cols x2
    add("molb", 2)
    add("id0", 256)          # [I128 | 0]
    add("id1", 256)          # [0 | I128]
    add("w1T", 2 * 128)      # mlp w1.T chunks
    add("b1", 1)
    add("w2T", 64)
    add("b2_", 1)
    return L, off


def make_wpack(inp):
    """Host: build wpack [128, WCOLS] f32 from the model inputs dict."""
    L, total = wpack_layout()
    W = np.zeros((128, total), np.float32)
    def put(name, arr):
        off, cols = L[name]
        assert arr.shape == (128, cols), (name, arr.shape, cols)
        W[:, off:off + cols] = arr
    put("iota_sq", np.tile(np.arange(128, dtype=np.float32), (128, 1)))
    put("iota256", np.tile(np.arange(256, dtype=np.float32), (128, 1)))
    put("iota_col", np.arange(128, dtype=np.float32).reshape(128, 1))
    put("ones_col", np.ones((128, 1), np.float32))
    put("attl_sq", np.tile(inp['g_att_l'], (128, 1)))
    put("attm_sq", np.tile(inp['mol_att'], (128, 1)))
    for l in range(3):
        put(f"att{l}_sq", np.tile(inp['atom_att'][l], (128, 1)))
    W1 = inp['g_lin1_w'][:, :256]
    W1T = W1.T.astype(np.float32)                      # [256 k, 256 h']
    put("W1T", np.concatenate([W1T[0:128], W1T[128:256]], axis=1))
    Wls = [inp['atom_Wl'][0], inp['atom_Wl'][1], inp['atom_Wl'][2], inp['mol_Wl']]
    Wrs = [inp['atom_Wr'][0], inp['atom_Wr'][1], inp['atom_Wr'][2], inp['mol_Wr']]
    for i in range(4):
        WT = Wls[i].T.astype(np.float32)
        put(f"WlTr{i}", np.concatenate([WT[0:128], WT[128:256]], axis=1))
        WT = Wrs[i].T.astype(np.float32)
        put(f"WrTr{i}", np.concatenate([WT[0:128], WT[128:256]], axis=1))
        for k in range(2):
            for b in range(2):
                put(f"WrTl{i}_{k}{b}", WT[k * 128:(k + 1) * 128, b * 128:(b + 1) * 128])
    g2T = inp['g_lin2_w'].T.astype(np.float32)         # [h k, h' m]
    for k in range(2):
        for b in range(2):
            put(f"gl2T_{k}{b}", g2T[k * 128:(k + 1) * 128, b * 128:(b + 1) * 128])
    put("gb2", inp['g_bias'].reshape(2, 128).T.astype(np.float32))
    put("attr_col", inp['g_att_r'].reshape(2, 128).T.astype(np.float32))
    grus = [('gru0_wih', 'gru0_whh', 'gru0_bih', 'gru0_bhh', None),
            ('agru_wih', 'agru_whh', 'agru_bih', 'agru_bhh', 0),
            ('agru_wih', 'agru_whh', 'agru_bih', 'agru_bhh', 1),
            ('agru_wih', 'agru_whh', 'agru_bih', 'agru_bhh', 2),
            ('mgru_wih', 'mgru_whh', 'mgru_bih', 'mgru_bhh', None)]
    for g, (wi, wh, bi, bh, l) in enumerate(grus):
        wih = inp[wi] if l is None else inp[wi][l]     # [768, 256]
        whh = inp[wh] if l is None else inp[wh][l]
        bih = inp[bi] if l is None else inp[bi][l]
        bhh = inp[bh] if l is None else inp[bh][l]
        # gates rows: r 0:256, z 256:512, n 512:768
        wihT = wih.T.astype(np.float32)                # [256 k, 768]
        whhT = whh.T.astype(np.float32)
        # j layout: r: 0,1 h-side kchunks; 2,3 x-side; z: 4..7; inn(h): 8,9; hn(x): 10,11
        for k in range(2):
            for b in range(2):
                ks, bs = slice(k * 128, (k + 1) * 128), slice(b * 128, (b + 1) * 128)
                put(f"gru{g}_w{0 + k}{b}", wihT[ks, 0:256][:, bs])
                put(f"gru{g}_w{2 + k}{b}", whhT[ks, 0:256][:, bs])
                put(f"gru{g}_w{4 + k}{b}", wihT[ks, 256:512][:, bs])
                put(f"gru{g}_w{6 + k}{b}", whhT[ks, 256:512][:, bs])
                put(f"gru{g}_w{8 + k}{b}", wihT[ks, 512:768][:, bs])
                put(f"gru{g}_w{10 + k}{b}", whhT[ks, 512:768][:, bs])
        br = (bih[0:256] + bhh[0:256]).reshape(2, 128).T
        bz = (bih[256:512] + bhh[256:512]).reshape(2, 128).T
        bin_ = bih[512:768].reshape(2, 128).T
        bhn = bhh[512:768].reshape(2, 128).T
        for j, arr in enumerate([br, bz, bin_, bhn]):
            for b in range(2):
                put(f"gru{g}_b{j}{b}", arr[:, b:b + 1].astype(np.float32))
    for i in range(3):
        put(f"ab{i}", inp['atom_bias'][i].reshape(2, 128).T.astype(np.float32))
    put("molb", inp['mol_bias'].reshape(2, 128).T.astype(np.float32))
    I = np.eye(128, dtype=np.float32)
    put("id0", np.concatenate([I, np.zeros((128, 128), np.float32)], 1))
    put("id1", np.concatenate([np.zeros((128, 128), np.float32), I], 1))
    w1T = inp['mlp_w1'].T.astype(np.float32)           # [256, 128]
    put("w1T", np.concatenate([w1T[0:128], w1T[128:256]], 1))
    put("b1", inp['mlp_b1'].reshape(128, 1).astype(np.float32))
    put("w2T", inp['mlp_w2'].T.astype(np.float32))     # [128, 64]
    put("b2_", np.zeros((128, 1), np.float32) + np.pad(inp['mlp_b2'], (0, 64)).reshape(128, 1))
    return W


def build_kernel(NLOC, NWIN, CW, NG, n_cores, taps=(), dt_tab=F32, stop_after=None):
    H = 256
    NCH = NWIN * CW
    assert NG in (128, 256)
    NGB = NG // 128

    nc = bass.Bass(num_devices=n_cores)
    L, WCOLS = wpack_layout()

    def dram_in(name, shape, dt=F32):
        return nc.dram_tensor(name, list(shape), dt, kind="ExternalInput")

    xinT = dram_in("xinT", [65, NLOC])
    srcblk = dram_in("srcblk", [NWIN, 128, CW], I32)
    dstlocblk = dram_in("dstlocblk", [NWIN, 128, CW])
    dstrowblk = dram_in("dstrowblk", [NWIN, 1, CW * 128])
    glocblk = dram_in("glocblk", [NWIN, 128, 1])
    eaTd = dram_in("eaT", [NCH, 16, 128])
    wpack = dram_in("wpack", [128, WCOLS])
    lin1Td = dram_in("lin1T", [65, 256])
    W2Td = dram_in("W2T", [16, 256])
    w3Td = dram_in("w3T", [65, 1])

    y = nc.dram_tensor("y", [1, 256], F32, kind="ExternalOutput")

    xT_a = nc.dram_tensor("xT_a", [2, 128, NLOC], F32)
    xT_b = nc.dram_tensor("xT_b", [2, 128, NLOC], F32)
    w_c = nc.dram_tensor("w_c", [NWIN, 128, 1], F32)
    cc_in = nc.dram_tensor("cc_in", [NLOC, H], dt_tab)
    tab_full = nc.dram_tensor("tab_full", [n_cores * NLOC, H], dt_tab, addr_space="Shared")
    hr_row = nc.dram_tensor("hr_row", [NLOC, H], dt_tab)
    x_row = nc.dram_tensor("x_row", [NLOC, H], dt_tab)
    hl_md = nc.dram_tensor("hl_m", [NLOC, H], dt_tab)

    dbg = {}
    for t in taps:
        shp = [3, 128, NLOC] if t.startswith('ags') else [2, 128, NLOC]
        dbg[t] = nc.dram_tensor(f"dbg_{t}", shp, F32, kind="ExternalOutput")

    with TileContextFixed(nc) as tc, ExitStack() as ctx:
        wpool = ctx.enter_context(tc.tile_pool(name="weights", bufs=1))
        cpool = ctx.enter_context(tc.tile_pool(name="chunk", bufs=2))
        spool = ctx.enter_context(tc.tile_pool(name="small", bufs=3))
        npool = ctx.enter_context(tc.tile_pool(name="node", bufs=2))
        gpool = ctx.enter_context(tc.tile_pool(name="grup", bufs=1))
        molpool = ctx.enter_context(tc.tile_pool(name="molp", bufs=1))
        mpool = ctx.enter_context(tc.tile_pool(name="meta", bufs=2))
        pp_chunk = ctx.enter_context(tc.tile_pool(name="pschunk", bufs=2, space="PSUM"))
        pp_acc = ctx.enter_context(tc.tile_pool(name="psacc", bufs=1, space="PSUM"))
        pp_gru = ctx.enter_context(tc.tile_pool(name="psgru", bufs=2, space="PSUM"))
        pp_misc = ctx.enter_context(tc.tile_pool(name="psmisc", bufs=1, space="PSUM"))

        wp = wpool.tile([128, WCOLS], F32, tag="wp")
        nc.sync.dma_start(wp[:], wpack.ap())
        def W(name):
            off, cols = L[name]
            return wp[:, off:off + cols]
        iota_sq, iota256 = W("iota_sq"), W("iota256")
        ones_col = W("ones_col")
        ident = W("id0")[:, 0:128]
        ones1 = wpool.tile([1, 128], F32, tag="ones1")
        nc.vector.memset(ones1[:], 1.0)
        lin1T = wpool.tile([65, 256], F32, tag="lin1T")
        nc.sync.dma_start(lin1T[:], lin1Td.ap())
        W2T = wpool.tile([16, 256], F32, tag="W2T")
        nc.sync.dma_start(W2T[:], W2Td.ap())
        w3T = wpool.tile([65, 1], F32, tag="w3T")
        nc.sync.dma_start(w3T[:], w3Td.ap())

        def misc_ps(cols=512):
            return pp_misc.tile([128, cols], F32, tag="misc", name="miscps")

        def tap(name, xT_cur):
            if name in dbg:
                tt = npool.tile([128, 256], F32, tag="tapt")
                for b in range(2):
                    for w in range(NWIN):
                        sl = slice(w * 128, (w + 1) * 128)
                        nc.sync.dma_start(tt[:, 0:128], xT_cur.ap()[b][:, sl])
                        nc.sync.dma_start(dbg[name].ap()[b][:, sl], tt[:, 0:128])

        def elu(out_ap, x_tile, pool, wcols):
            u = pool.tile([128, wcols], F32, tag=f"elu_u{wcols}", name="eluu")
            nc.vector.tensor_scalar(out=u[:], in0=x_tile[:], scalar1=0.0,
                                    scalar2=None, op0=alu.min)
            eu = pool.tile([128, wcols], F32, tag=f"elu_e{wcols}", name="elue")
            nc.scalar.activation(eu[:], u[:], act.Exp)
            t = pool.tile([128, wcols], F32, tag=f"elu_t{wcols}", name="elut")
            nc.vector.scalar_tensor_tensor(out=t[:], in0=u[:], scalar=-1.0, in1=eu[:],
                                           op0=alu.mult, op1=alu.add)
            nc.vector.scalar_tensor_tensor(out=out_ap, in0=x_tile[:], scalar=-1.0,
                                           in1=t[:], op0=alu.add, op1=alu.add)

        def gru(g, hT, xT, n, WIDE, relu_out=True):
            """transposed gru: hT/xT [128, 2*WIDE]; returns xn [128, 2*WIDE]."""
            xn = gpool.tile([128, 2 * WIDE], F32, tag=f"gru_xn{WIDE}", name="gruxn")
            for b in range(2):
                bs = slice(b * WIDE, (b + 1) * WIDE)
                def gate_mm(ps, joff_h, joff_x):
                    for k in range(2):
                        ks = slice(k * WIDE, (k + 1) * WIDE)
                        if joff_h is not None:
                            nc.tensor.matmul(ps[:], lhsT=W(f"gru{g}_w{joff_h + k}{b}"),
                                             rhs=hT[:, ks], start=(k == 0),
                                             stop=(k == 1 and joff_x is None))
                        if joff_x is not None:
                            nc.tensor.matmul(ps[:], lhsT=W(f"gru{g}_w{joff_x + k}{b}"),
                                             rhs=xT[:, ks],
                                             start=(k == 0 and joff_h is None),
                                             stop=(k == 1))
                rps = pp_gru.tile([128, WIDE], F32, tag="gp", name="rps")
                gate_mm(rps, 0, 2)
                r = gpool.tile([128, WIDE], F32, tag=f"gru_r{WIDE}", name="grur")
                nc.scalar.activation(r[:], rps[:], act.Sigmoid, bias=W(f"gru{g}_b0{b}"))
                zps = pp_gru.tile([128, WIDE], F32, tag="gp", name="zps")
                gate_mm(zps, 4, 6)
                z = gpool.tile([128, WIDE], F32, tag=f"gru_z{WIDE}", name="gruz")
                nc.scalar.activation(z[:], zps[:], act.Sigmoid, bias=W(f"gru{g}_b1{b}"))
                ips = pp_gru.tile([128, WIDE], F32, tag="gp", name="ips")
                gate_mm(ips, 8, None)
                hps = pp_gru.tile([128, WIDE], F32, tag="gp", name="hps")
                gate_mm(hps, None, 10)
                t1 = gpool.tile([128, WIDE], F32, tag=f"gru_t1{WIDE}", name="grut1")
                nc.vector.scalar_tensor_tensor(out=t1[:], in0=hps[:],
                                               scalar=W(f"gru{g}_b3{b}"), in1=r[:],
                                               op0=alu.add, op1=alu.mult)
                t2 = gpool.tile([128, WIDE], F32, tag=f"gru_t2{WIDE}", name="grut2")
                nc.vector.tensor_tensor(out=t2[:], in0=t1[:], in1=ips[:], op=alu.add)
                nn_ = gpool.tile([128, WIDE], F32, tag=f"gru_n{WIDE}", name="grun")
                nc.scalar.activation(nn_[:], t2[:], act.Tanh, bias=W(f"gru{g}_b2{b}"))
                d = gpool.tile([128, WIDE], F32, tag=f"gru_d{WIDE}", name="grud")
                nc.vector.tensor_tensor(out=d[:], in0=xT[:, bs], in1=nn_[:], op=alu.subtract)
                zd = gpool.tile([128, WIDE], F32, tag=f"gru_zd{WIDE}", name="gruzd")
                nc.vector.tensor_tensor(out=zd[:], in0=z[:], in1=d[:], op=alu.mult)
                if relu_out:
                    t3 = gpool.tile([128, WIDE], F32, tag=f"gru_t3{WIDE}", name="grut3")
                    nc.vector.tensor_tensor(out=t3[:], in0=nn_[:], in1=zd[:], op=alu.add)
                    nc.scalar.activation(xn[:, bs], t3[:], act.Relu)
                else:
                    nc.vector.tensor_tensor(out=xn[:, bs], in0=nn_[:], in1=zd[:], op=alu.add)
            return xn

        # ================= P0: projection =================
        for w in range(NWIN):
            sl = slice(w * 128, (w + 1) * 128)
            xin_t = mpool.tile([65, 128], F32, tag="xin", bufs=1)
            nc.sync.dma_start(xin_t[:], xinT.ap()[:, sl])
            x0ps = misc_ps(256)
            for b in range(2):
                nc.tensor.matmul(x0ps[:, b * 128:(b + 1) * 128],
                                 lhsT=lin1T[:, b * 128:(b + 1) * 128],
                                 rhs=xin_t[:], start=True, stop=True)
            x0T = npool.tile([128, 256], F32, tag="x0T")
            for b in range(2):
                nc.scalar.activation(x0T[:, b * 128:(b + 1) * 128],
                                     x0ps[:, b * 128:(b + 1) * 128], act.Lrelu, alpha=0.01)
                nc.sync.dma_start(xT_a.ap()[b][:, sl], x0T[:, b * 128:(b + 1) * 128])
            ups = pp_acc.tile([128, 1536], F32, tag="aggps", name="ups")
            for b in range(2):
                nc.tensor.matmul(ups[:, 0:256], lhsT=x0T[:, b * 128:(b + 1) * 128],
                                 rhs=W(f"W1T")[:, b * 256:(b + 1) * 256],
                                 start=(b == 0), stop=(b == 1))
            for b in range(2):
                nc.tensor.matmul(ups[:, 512:513], lhsT=x0T[:, b * 128:(b + 1) * 128],
                                 rhs=W("attr_col")[:, b:b + 1],
                                 start=(b == 0), stop=(b == 1))
            u_sb = npool.tile([128, 257], dt_tab, tag="tabsb", name="tabsb")
            nc.vector.tensor_copy(u_sb[:, 0:256], ups[:, 0:256])
            nc.vector.tensor_copy(u_sb[:, 256:257], ups[:, 512:513])
            nc.sync.dma_start(cc_in.ap()[sl, :], u_sb[:, 0:256])
            nc.sync.dma_start(w_c.ap()[w], u_sb[:, 256:257])

        def allgather():
            if n_cores == 1:
                nc.sync.dma_start(tab_full.ap()[:, :], cc_in.ap()[:, :])
            else:
                nc.gpsimd.collective_compute(
                    "AllGather", alu.bypass,
                    replica_groups=[list(range(n_cores))],
                    ins=[cc_in.ap()], outs=[tab_full.ap()])
        allgather()

        # ================= edge layers =================
        def edge_layer(kind, l, xT_src, xT_dst, last=False):
            gru_i = 0 if kind == 'gate' else 1 + l
            attw = W("attl_sq") if kind == 'gate' else W(f"att{l}_sq")
            for w in range(NWIN):
                sl = slice(w * 128, (w + 1) * 128)
                srct = mpool.tile([128, CW], I32, tag="srct")
                nc.sync.dma_start(srct[:], srcblk.ap()[w])
                dlc = mpool.tile([128, CW], F32, tag="dlc")
                nc.sync.dma_start(dlc[:], dstlocblk.ap()[w])
                drow = mpool.tile([1, CW * 128], F32, tag="drow", bufs=1)
                nc.sync.dma_start(drow[:], dstrowblk.ap()[w])
                xTw = npool.tile([128, 256], F32, tag="xTw")
                for b in range(2):
                    nc.sync.dma_start(xTw[:, b * 128:(b + 1) * 128], xT_src.ap()[b][:, sl])
                if kind == 'atom':
                    hrw = npool.tile([128, H], dt_tab, tag="hrw")
                    nc.sync.dma_start(hrw[:], hr_row.ap()[sl, :])
                    hrT = npool.tile([128, 256], F32, tag="hrT")
                    hrTps = misc_ps(256)
                    for b in range(2):
                        for k in range(2):
                            nc.tensor.matmul(hrTps[:, b * 128:(b + 1) * 128],
                                             lhsT=W(f"WrTl{l}_{k}{b}"),
                                             rhs=xTw[:, k * 128:(k + 1) * 128],
                                             start=(k == 0), stop=(k == 1))
                    nc.vector.tensor_copy(hrT[:], hrTps[:])
                else:
                    wwin = spool.tile([128, 1], F32, tag="wwin")
                    nc.sync.dma_start(wwin[:], w_c.ap()[w])

                aggps = pp_acc.tile([128, 1536], F32, tag="aggps", name="aggps")
                AGG = [0, 512]  # col offset of agg block b (separate banks)
                SCOL, RCOL = 1024, 1152

                for ci in range(CW):
                    first, lastc = (ci == 0), (ci == CW - 1)
                    chps = pp_chunk.tile([128, 512], F32, tag="chps")
                    # dst_bcast [:,256:384]
                    nc.tensor.matmul(chps[:, 256:384], lhsT=ones1[:],
                                     rhs=drow[:, ci * 128:(ci + 1) * 128],
                                     start=True, stop=True)
                    selT = spool.tile([128, 128], F32, tag="selT")
                    nc.vector.tensor_scalar(out=selT[:], in0=chps[:, 256:384],
                                            scalar1=W("iota_col"), scalar2=None,
                                            op0=alu.is_equal)
                    g = cpool.tile([128, H], dt_tab, tag="gather", bufs=4)
                    nc.gpsimd.indirect_dma_start(
                        out=g[:], out_offset=None, in_=tab_full.ap(),
                        in_offset=IndirectOffsetOnAxis(ap=srct[:, ci:ci + 1], axis=0))
                    if kind == 'atom':
                        nc.tensor.matmul(chps[:, 0:256], lhsT=selT[:], rhs=hrw[:],
                                         start=True, stop=True)
                    else:
                        eat = mpool.tile([16, 128], F32, tag="eat", bufs=1)
                        nc.sync.dma_start(eat[:], eaTd.ap()[w * CW + ci])
                        nc.tensor.matmul(chps[:, 0:256], lhsT=eat[:], rhs=W2T[:],
                                         start=True, stop=True)
                        nc.tensor.matmul(chps[:, 384:385], lhsT=selT[:], rhs=wwin[:],
                                         start=True, stop=True)
                    t_t = cpool.tile([128, 256], F32, tag="t_t")
                    nc.vector.scalar_tensor_tensor(out=t_t[:], in0=chps[:, 0:256],
                                                   scalar=0.0, in1=g[:],
                                                   op0=alu.add, op1=alu.add)
                    tl = cpool.tile([128, 256], F32, tag="tl")
                    nc.scalar.activation(tl[:], t_t[:], act.Lrelu, alpha=0.01)
                    escr = cpool.tile([128, 256], F32, tag="escr")
                    ecol = spool.tile([128, 1], F32, tag="ecol")
                    nc.vector.tensor_tensor(out=escr[:], in0=tl[:], in1=attw, op=alu.mult)
                    nc.vector.reduce_sum(out=ecol[:], in_=escr[:], axis=mybir.AxisListType.X)
                    ex = spool.tile([128, 1], F32, tag="ex")
                    if kind == 'gate':
                        e2 = spool.tile([128, 1], F32, tag="e2")
                        nc.vector.scalar_tensor_tensor(out=e2[:], in0=chps[:, 384:385],
                                                       scalar=0.0, in1=ecol[:],
                                                       op0=alu.add, op1=alu.add)
                        el = spool.tile([128, 1], F32, tag="el")
                        nc.scalar.activation(el[:], e2[:], act.Lrelu, alpha=0.01)
                        nc.scalar.activation(ex[:], el[:], act.Exp)
                    else:
                        nc.scalar.activation(ex[:], ecol[:], act.Exp)
                    selw = spool.tile([128, 128], F32, tag="selw")
                    nc.vector.scalar_tensor_tensor(out=selw[:], in0=iota_sq,
                                                   scalar=dlc[:, ci:ci + 1],
                                                   op0=alu.is_equal, op1=alu.mult,
                                                   in1=ex[:].to_broadcast([128, 128]))
                    val = t_t if kind == 'atom' else tl
                    for b in range(2):
                        nc.tensor.matmul(aggps[:, AGG[b]:AGG[b] + 128],
                                         lhsT=val[:, b * 128:(b + 1) * 128],
                                         rhs=selw[:], start=first, stop=lastc)
                    nc.tensor.matmul(aggps[0:1, SCOL:SCOL + 128], lhsT=ones_col, rhs=selw[:],
                                     start=first, stop=lastc)

                # ---- epilogue ----
                sraw = spool.tile([1, 128], F32, tag="sraw")
                nc.vector.tensor_copy(sraw[:], aggps[0:1, SCOL:SCOL + 128])
                srow = spool.tile([1, 128], F32, tag="srow")
                nc.vector.tensor_scalar(out=srow[:], in0=sraw[:],
                                        scalar1=EPS, scalar2=None, op0=alu.max)
                rrow = spool.tile([1, 128], F32, tag="rrow")
                nc.vector.reciprocal(rrow[:], srow[:])
                nc.tensor.matmul(aggps[:, RCOL:RCOL + 128], lhsT=ones1[:], rhs=rrow[:],
                                 start=True, stop=True)
                rbc = npool.tile([128, 128], F32, tag="rbc")
                nc.vector.tensor_copy(rbc[:], aggps[:, RCOL:RCOL + 128])
                hT = npool.tile([128, 256], F32, tag="hT")
                if kind == 'atom':
                    sps = misc_ps(128)
                    nc.tensor.matmul(sps[:, 0:128], lhsT=ones1[:],
                                     rhs=sraw[:], start=True, stop=True)
                    sbc = npool.tile([128, 128], F32, tag="sbc")
                    nc.vector.tensor_copy(sbc[:], sps[:, 0:128])
                    for b in range(2):
                        bs = slice(b * 128, (b + 1) * 128)
                        t1 = npool.tile([128, 128], F32, tag="ep_t1")
                        nc.vector.scalar_tensor_tensor(out=t1[:], in0=hrT[:, bs],
                                                       scalar=-1.0, in1=sbc[:],
                                                       op0=alu.mult, op1=alu.mult)
                        nc.vector.tensor_tensor(out=t1[:], in0=t1[:],
                                                in1=aggps[:, AGG[b]:AGG[b] + 128],
                                                op=alu.add)
                        hpre = npool.tile([128, 128], F32, tag="ep_hpre")
                        nc.vector.tensor_tensor(out=hpre[:], in0=t1[:], in1=rbc[:],
                                                op=alu.mult)
                        nc.vector.tensor_scalar(out=hpre[:], in0=hpre[:],
                                                scalar1=W(f"ab{l}")[:, b:b + 1],
                                                scalar2=None, op0=alu.add)
                        if f'cor1_{kind}{l}' in dbg:
                            nc.sync.dma_start(dbg[f'cor1_{kind}{l}'].ap()[b][:, sl], t1[:])
                            nc.sync.dma_start(dbg[f'cor2_{kind}{l}'].ap()[b][:, sl], hpre[:])
                        elu(hT[:, bs], hpre, npool, 128)
                else:
                    aggn = npool.tile([128, 256], F32, tag="aggn")
                    for b in range(2):
                        bs = slice(b * 128, (b + 1) * 128)
                        nc.vector.tensor_tensor(out=aggn[:, bs],
                                                in0=aggps[:, AGG[b]:AGG[b] + 128],
                                                in1=rbc[:], op=alu.mult)
                    h0ps = misc_ps(256)
                    for b in range(2):
                        for k in range(2):
                            nc.tensor.matmul(h0ps[:, b * 128:(b + 1) * 128],
                                             lhsT=W(f"gl2T_{k}{b}"),
                                             rhs=aggn[:, k * 128:(k + 1) * 128],
                                             start=(k == 0), stop=(k == 1))
                    for b in range(2):
                        bs = slice(b * 128, (b + 1) * 128)
                        hpre = npool.tile([128, 128], F32, tag="ep_hpre")
                        nc.vector.tensor_scalar(out=hpre[:], in0=h0ps[:, bs],
                                                scalar1=W("gb2")[:, b:b + 1],
                                                scalar2=None, op0=alu.add)
                        elu(hT[:, bs], hpre, npool, 128)

                tkey = f'hrt_{kind}{l}'
                if tkey in dbg and kind == 'atom':
                    for b in range(2):
                        nc.sync.dma_start(dbg[tkey].ap()[b][:, sl], hrT[:, b * 128:(b + 1) * 128])
                hkey = f'h_{kind}{l}'
                if hkey in dbg:
                    for b in range(2):
                        nc.sync.dma_start(dbg[hkey].ap()[b][:, sl], hT[:, b * 128:(b + 1) * 128])
                akey = f'ags_{kind}{l}'
                if akey in dbg:
                    agt = npool.tile([128, 256], F32, tag="agt")
                    for b in range(2):
                        nc.vector.tensor_copy(agt[:, b * 128:(b + 1) * 128],
                                              aggps[:, AGG[b]:AGG[b] + 128])
                        nc.sync.dma_start(dbg[akey].ap()[b][:, sl], agt[:, b * 128:(b + 1) * 128])
                    nc.sync.dma_start(dbg[akey].ap()[2][0:1, sl], srow[:])
                xnT = gru(gru_i, hT, xTw, 256, 128)
                for b in range(2):
                    nc.sync.dma_start(xT_dst.ap()[b][:, sl], xnT[:, b * 128:(b + 1) * 128])

                if not last:
                    nl = (l + 1) if kind == 'atom' else 0
                    hlps = misc_ps(256)
                    for k in range(2):
                        nc.tensor.matmul(hlps[:, 0:256], lhsT=xnT[:, k * 128:(k + 1) * 128],
                                         rhs=W(f"WlTr{nl}")[:, k * 256:(k + 1) * 256],
                                         start=(k == 0), stop=(k == 1))
                    hlsb = npool.tile([128, H], dt_tab, tag="tabsb", name="tabsb")
                    nc.vector.tensor_copy(hlsb[:], hlps[:, 0:256])
                    nc.sync.dma_start(cc_in.ap()[sl, :], hlsb[:])
                    hrps = misc_ps(256)
                    for k in range(2):
                        nc.tensor.matmul(hrps[:, 0:256], lhsT=xnT[:, k * 128:(k + 1) * 128],
                                         rhs=W(f"WrTr{nl}")[:, k * 256:(k + 1) * 256],
                                         start=(k == 0), stop=(k == 1))
                    hrsb = npool.tile([128, H], dt_tab, tag="tabsb", name="tabsb")
                    nc.vector.tensor_copy(hrsb[:], hrps[:, 0:256])
                    nc.sync.dma_start(hr_row.ap()[sl, :], hrsb[:])
                else:
                    xrps = misc_ps(256)
                    for k in range(2):
                        nc.tensor.matmul(xrps[:, 0:256], lhsT=xnT[:, k * 128:(k + 1) * 128],
                                         rhs=W("id0") if k == 0 else W("id1"),
                                         start=(k == 0), stop=(k == 1))
                    xrsb = npool.tile([128, H], dt_tab, tag="tabsb", name="tabsb")
                    nc.vector.tensor_copy(xrsb[:], xrps[:, 0:256])
                    nc.sync.dma_start(x_row.ap()[sl, :], xrsb[:])
                    hmps = misc_ps(256)
                    for k in range(2):
                        nc.tensor.matmul(hmps[:, 0:256], lhsT=xnT[:, k * 128:(k + 1) * 128],
                                         rhs=W("WlTr3")[:, k * 256:(k + 1) * 256],
                                         start=(k == 0), stop=(k == 1))
                    hmsb = npool.tile([128, H], dt_tab, tag="tabsb", name="tabsb")
                    nc.vector.tensor_copy(hmsb[:], hmps[:, 0:256])
                    nc.sync.dma_start(hl_md.ap()[sl, :], hmsb[:])

        edge_layer('gate', 0, xT_a, xT_b)
        tap('x1', xT_b)
        if stop_after != 'x1':
            allgather()
            edge_layer('atom', 0, xT_b, xT_a)
            tap('x2', xT_a)
        if stop_after not in ('x1', 'x2'):
            allgather()
            edge_layer('atom', 1, xT_a, xT_b)
            allgather()
            edge_layer('atom', 2, xT_b, xT_a, last=True)
            tap('x4', xT_a)

        if stop_after in ('x1', 'x2'):
            ob = spool.tile([1, 256], F32, tag="ob")
            nc.vector.memset(ob[:], 0.0)
            nc.sync.dma_start(y.ap()[:, :], ob[:])
            return nc
        # ================= mol phase =================
        glc_cache = []
        for w in range(NWIN):
            t = wpool.tile([128, 1], F32, tag=f"glcc_{w}", name="glcc")
            nc.sync.dma_start(t[:], glocblk.ap()[w])
            glc_cache.append(t)

        ro_ps = pp_acc.tile([128, 1536], F32, tag="aggps", name="rops")
        for w in range(NWIN):
            xr = cpool.tile([128, H], dt_tab, tag="xr")
            nc.sync.dma_start(xr[:], x_row.ap()[w * 128:(w + 1) * 128, :])
            selg = npool.tile([128, NG], F32, tag="selg")
            nc.vector.tensor_scalar(out=selg[:], in0=iota256[:, 0:NG],
                                    scalar1=glc_cache[w][:], scalar2=None,
                                    op0=alu.is_equal)
            for b in range(2):
                nc.tensor.matmul(ro_ps[:, b * 512:b * 512 + NG],
                                 lhsT=xr[:, b * 128:(b + 1) * 128], rhs=selg[:],
                                 start=(w == 0), stop=(w == NWIN - 1))
        outT = wpool.tile([128, 2 * NG], F32, tag="outT0")
        for b in range(2):
            nc.scalar.activation(outT[:, b * NG:(b + 1) * NG],
                                 ro_ps[:, b * 512:b * 512 + NG], act.Relu)

        for step in range(3):
            # hr_m rows [NG, 256] and hrmT [128, 2*NG]
            hrm = molpool.tile([128, NGB * 256], F32, tag="hrm")
            for gb in range(NGB):
                hrps = misc_ps(256)
                for k in range(2):
                    nc.tensor.matmul(
                        hrps[:, 0:256],
                        lhsT=outT[:, k * NG + gb * 128: k * NG + gb * 128 + 128],
                        rhs=W("WrTr3")[:, k * 256:(k + 1) * 256],
                        start=(k == 0), stop=(k == 1))
                nc.vector.tensor_copy(hrm[:, gb * 256:(gb + 1) * 256], hrps[:, 0:256])
            hrmT = molpool.tile([128, 2 * NG], F32, tag="hrmT")
            for b in range(2):
                hrmTps = pp_gru.tile([128, NG], F32, tag="gp", name="hrmTps")
                for k in range(2):
                    nc.tensor.matmul(hrmTps[:],
                                     lhsT=W(f"WrTl3_{k}{b}"),
                                     rhs=outT[:, k * NG:(k + 1) * NG],
                                     start=(k == 0), stop=(k == 1))
                nc.vector.tensor_copy(hrmT[:, b * NG:(b + 1) * NG], hrmTps[:])

            agm_t = pp_acc.tile([128, 1536], F32, tag="aggps", name="agmt")
            sg_ps = agm_t[0:1, 1024:1024 + NG]
            for w in range(NWIN):
                selg = npool.tile([128, NG], F32, tag="selg")
                nc.vector.tensor_scalar(out=selg[:], in0=iota256[:, 0:NG],
                                        scalar1=glc_cache[w][:], scalar2=None,
                                        op0=alu.is_equal)
                Bps = pp_chunk.tile([128, 512], F32, tag="chps")
                for gb in range(NGB):
                    tps = misc_ps(128)
                    nc.tensor.transpose(out=tps[:, 0:128],
                                        in_=selg[:, gb * 128:(gb + 1) * 128],
                                        identity=ident)
                    sTg = spool.tile([128, 128], F32, tag="sTg")
                    nc.vector.tensor_copy(sTg[:], tps[:, 0:128])
                    nc.tensor.matmul(Bps[:, 0:256], lhsT=sTg[:],
                                     rhs=hrm[:, gb * 256:(gb + 1) * 256],
                                     start=(gb == 0), stop=(gb == NGB - 1))
                hmw = cpool.tile([128, H], dt_tab, tag="hmw")
                nc.sync.dma_start(hmw[:], hl_md.ap()[w * 128:(w + 1) * 128, :])
                tt = cpool.tile([128, 256], F32, tag="ttm")
                nc.vector.scalar_tensor_tensor(out=tt[:], in0=Bps[:, 0:256], scalar=0.0,
                                               in1=hmw[:], op0=alu.add, op1=alu.add)
                tlm = cpool.tile([128, 256], F32, tag="tlm")
                nc.scalar.activation(tlm[:], tt[:], act.Lrelu, alpha=0.01)
                escr = cpool.tile([128, 256], F32, tag="escr")
                ecol = spool.tile([128, 1], F32, tag="ecol")
                nc.vector.tensor_tensor(out=escr[:], in0=tlm[:], in1=W("attm_sq"), op=alu.mult)
                nc.vector.reduce_sum(out=ecol[:], in_=escr[:], axis=mybir.AxisListType.X)
                exm = spool.tile([128, 1], F32, tag="ex")
                nc.scalar.activation(exm[:], ecol[:], act.Exp)
                selwm = npool.tile([128, NG], F32, tag="selwm")
                nc.vector.tensor_tensor(out=selwm[:], in0=selg[:],
                                        in1=exm[:].to_broadcast([128, NG]), op=alu.mult)
                for b in range(2):
                    nc.tensor.matmul(agm_t[:, b * 512:b * 512 + NG],
                                     lhsT=tt[:, b * 128:(b + 1) * 128], rhs=selwm[:],
                                     start=(w == 0), stop=(w == NWIN - 1))
                nc.tensor.matmul(sg_ps, lhsT=ones_col, rhs=selwm[:],
                                 start=(w == 0), stop=(w == NWIN - 1))

            smraw = spool.tile([1, NG], F32, tag="smraw")
            nc.vector.tensor_copy(smraw[:], sg_ps)
            srowm = spool.tile([1, NG], F32, tag="srowm")
            nc.vector.tensor_scalar(out=srowm[:], in0=smraw[:], scalar1=EPS,
                                    scalar2=None, op0=alu.max)
            rrowm = spool.tile([1, NG], F32, tag="rrowm")
            nc.vector.reciprocal(rrowm[:], srowm[:])
            rsps = misc_ps(2 * NG)
            nc.tensor.matmul(rsps[:, 0:NG], lhsT=ones1[:], rhs=rrowm[:],
                             start=True, stop=True)
            nc.tensor.matmul(rsps[:, NG:2 * NG], lhsT=ones1[:], rhs=smraw[:],
                             start=True, stop=True)
            rbcm = molpool.tile([128, 2 * NG], F32, tag="rbcm")
            nc.vector.tensor_copy(rbcm[:], rsps[:, 0:2 * NG])
            hTm = molpool.tile([128, 2 * NG], F32, tag="hTm")
            for b in range(2):
                bs = slice(b * NG, (b + 1) * NG)
                t1 = molpool.tile([128, NG], F32, tag="ep_t1m")
                nc.vector.scalar_tensor_tensor(out=t1[:], in0=hrmT[:, bs], scalar=-1.0,
                                               in1=rbcm[:, NG:2 * NG],
                                               op0=alu.mult, op1=alu.mult)
                nc.vector.tensor_tensor(out=t1[:], in0=t1[:],
                                        in1=agm_t[:, b * 512:b * 512 + NG], op=alu.add)
                hpre = molpool.tile([128, NG], F32, tag="ep_hprem")
                nc.vector.tensor_tensor(out=hpre[:], in0=t1[:], in1=rbcm[:, 0:NG],
                                        op=alu.mult)
                nc.vector.tensor_scalar(out=hpre[:], in0=hpre[:],
                                        scalar1=W("molb")[:, b:b + 1],
                                        scalar2=None, op0=alu.add)
                elu(hTm[:, bs], hpre, molpool, NG)
            outT = gru(4, hTm, outT, 2 * NG, NG)

        # ================= MLP =================
        o1ps = misc_ps(NG)
        for k in range(2):
            nc.tensor.matmul(o1ps[:, 0:NG], lhsT=W("w1T")[:, k * 128:(k + 1) * 128],
                             rhs=outT[:, k * NG:(k + 1) * NG],
                             start=(k == 0), stop=(k == 1))
        o1 = npool.tile([128, NG], F32, tag="tabsb", name="o1t")
        nc.scalar.activation(o1[:], o1ps[:, 0:NG], act.Relu, bias=W("b1"))
        o2ps = pp_chunk.tile([64, NG], F32, tag="chps")
        nc.tensor.matmul(o2ps[:], lhsT=W("w2T"), rhs=o1[:], start=True, stop=True)
        o2 = npool.tile([65, NG], F32, tag="tabsb", name="o2t")
        nc.vector.memset(o2[64:65, :], 1.0)
        nc.scalar.activation(o2[0:64, :], o2ps[:], act.Relu, bias=W("b2_")[0:64, :])
        o3ps = pp_gru.tile([1, NG], F32, tag="gp", name="o3ps")
        nc.tensor.matmul(o3ps[:], lhsT=w3T[:], rhs=o2[:], start=True, stop=True)
        o3 = spool.tile([1, NG], F32, tag="o3")
        nc.vector.tensor_copy(o3[:], o3ps[:])
        nc.sync.dma_start(y.ap()[:, 0:NG], o3[:])

    return nc


def make_core_inputs(P, inputs, ci, dt_np=np.float32):
    """Host: per-core input arrays for core ci."""
    c = P['cores'][ci]
    NLOC = P['NLOC']
    x = np.asarray(inputs['x'], np.float32)
    xinT = np.zeros((65, NLOC), np.float32)
    xinT[:64, :c['nn']] = x[c['ns']:c['ne']].T
    xinT[64, :] = 1.0
    ea = np.asarray(inputs['edge_attr'], np.float32)
    ea_perm = np.where(c['sl_fill'][:, None], ea[c['sl_edge']], 0.0).astype(np.float32)
    NCH = P['NCH']
    eaT = ea_perm.reshape(NCH, 128, 16).transpose(0, 2, 1).copy()
    return dict(
        xinT=xinT,
        srcblk=c['srcblk'], dstlocblk=c['dstlocblk'], dstrowblk=c['dstrowblk'],
        glocblk=c['glocblk'], eaT=eaT,
        wpack=make_wpack(inputs),
        lin1T=np.concatenate([np.asarray(inputs['lin1_w'], np.float32).T,
                              np.asarray(inputs['lin1_b'], np.float32)[None, :]], 0),
        W2T=np.asarray(inputs['g_lin1_w'], np.float32)[:, 256:].T.copy(),
        w3T=np.concatenate([np.asarray(inputs['mlp_w3'], np.float32).T,
                            np.asarray(inputs['mlp_b3'], np.float32).reshape(1, 1)], 0),
    )


_CACHE = {}
LAST_EXEC_NS = None

def kernel(**inputs):
    inputs = dict(inputs)
    edge_index = np.asarray(inputs['edge_index']).astype(np.int64)
    batch = np.asarray(inputs['batch']).astype(np.int64)
    n_cores = 8
    G = 2048
    P = preprocess(edge_index, batch, n_cores=n_cores, G=G, CW=5)
    key = (P['NLOC'], P['NWIN'], P['CW'], P['GPC'])
    if key not in _CACHE:
        _CACHE[key] = build_kernel(P['NLOC'], P['NWIN'], P['CW'], P['GPC'], n_cores)
    nc = _CACHE[key]
    ins = [make_core_inputs(P, inputs, ci) for ci in range(n_cores)]
    from concourse.bass_utils import run_bass_kernel_spmd
    trace = bool(os.environ.get('BASS_KERNEL_TRACE'))
    res = run_bass_kernel_spmd(nc, ins, list(range(n_cores)), trace=trace)
    if trace:
        global LAST_EXEC_NS
        LAST_EXEC_NS = res.exec_time_ns
    y = np.concatenate([res.results[c]['y'][0, :P['GPC']] for c in range(n_cores)])
    return y.reshape(G, 1).astype(np.float32)



# revision 19
# speedup vs baseline: 1.9278x; 1.9278x over previous
"""AttentiveFP forward pass as a Bass/Tile kernel on 8 Trainium2 NeuronCores.

v2 optimizations over the fp32 baseline:
- bf16 matmuls (4x PE throughput) and bf16 tables/DMA (2x bytes)
- node-feature (xT) and hr tables SBUF-resident across layers (no DRAM round trips)
- zero steady-state ACT table reloads: only Exp/Tanh/Relu used (sigmoid via
  tanh identity, leaky-relu on DVE); P0 uses Lrelu once up front
- per-window batched DVE ops over all CW chunks at once
- one multi-index indirect gather per window (offset AP [128, CW])
- AllGather split into NCC groups, issued as their windows complete ->
  collective overlaps compute; gather tables double-buffered across layers
- softmax bias folded in as rank-1 (bias x s) matmul; elu computed as
  elu+1 = max(q,0)+exp(min(q,0)) with GRU input-bias compensation
- reciprocal via single-pass approx (51 ULP, plenty for softmax denom)

Layouts:
  xT tiles  [128, 256] per window: partition = h-in-block, col = k*128 + n
  hr tiles  [128, 256] per window: partition = node, col = h   (node-major)
  gather tab [NTAB, 256] bf16, rows grouped by (cc-group, core, window)
"""
import sys, os
sys.path.insert(0, '/opt/trn_rl_repo')
import numpy as np
from contextlib import ExitStack

import concourse.bass as bass
import concourse.mybir as mybir
import concourse.tile as tile
from concourse.bass import IndirectOffsetOnAxis
from concourse.mybir import AluOpType as alu, ActivationFunctionType as act

G_DEFAULT = 2048
F32 = mybir.dt.float32
BF16 = mybir.dt.bfloat16
I32 = mybir.dt.int32
EPS = 1e-30
NCC = 4   # collective groups per layer


def to_bf16(x):
    """np float32 -> np uint16 bf16 bits (round to nearest even)."""
    x = np.asarray(x, np.float32)
    u = x.view(np.uint32)
    lower = u & 0xffff
    upper = (u >> 16).astype(np.uint32)
    round_bit = (lower > 0x8000) | ((lower == 0x8000) & ((upper & 1) == 1))
    return (upper + round_bit).astype(np.uint16)


# ---------------- host preprocessing ----------------

def preprocess(edge_index, batch, n_cores=8, G=2048, CW=5):
    src = np.asarray(edge_index[0]).astype(np.int64)
    dst = np.asarray(edge_index[1]).astype(np.int64)
    batch = np.asarray(batch).astype(np.int64)
    N = batch.shape[0]
    GPC = G // n_cores
    gstart = np.searchsorted(batch, np.arange(0, G + 1, GPC))
    ncounts = np.diff(gstart)
    NLOC = int(np.ceil(ncounts.max() / 128) * 128)
    NWIN = NLOC // 128

    # collective groups: split windows into NCC groups
    gb = tuple(round(i * NWIN / NCC) for i in range(NCC + 1))
    nk = np.array([gb[i + 1] - gb[i] for i in range(NCC)])
    goff = np.concatenate([[0], np.cumsum(n_cores * nk * 128)])
    win_grp = np.zeros(NWIN, np.int64)
    for i in range(NCC):
        win_grp[gb[i]:gb[i + 1]] = i
    NTAB = int(goff[-1])
    gbl = np.array(gb)

    node_owner = np.searchsorted(gstart, np.arange(N), side='right') - 1
    loc = np.arange(N) - gstart[node_owner]
    wloc = loc // 128
    gi = win_grp[wloc]
    ag_row = (goff[gi] + node_owner * (nk[gi] * 128)
              + (loc - gbl[gi] * 128)).astype(np.int64)
    owner = node_owner[dst]

    cores = []
    for c in range(n_cores):
        ns, ne = int(gstart[c]), int(gstart[c + 1])
        nn = ne - ns
        m = owner == c
        eidx = np.nonzero(m)[0]
        dl = dst[eidx] - ns
        order = np.argsort(dl, kind='stable')
        eidx = eidx[order]; dl = dl[order]
        win = dl // 128
        counts = np.bincount(win, minlength=NWIN)
        assert counts.max() <= CW * 128, f"window overflow {counts.max()}"
        pos = np.concatenate([[0], np.cumsum(counts)])[:-1]
        within = np.arange(len(dl)) - pos[win]
        slots = (win * CW * 128 + within).astype(np.int64)

        NCH = NWIN * CW
        sl_src_ag = np.zeros(NCH * 128, np.int32)
        sl_dstloc = np.full(NCH * 128, -1.0, np.float32)
        sl_edge = np.zeros(NCH * 128, np.int64)
        sl_fill = np.zeros(NCH * 128, bool)
        sl_src_ag[slots] = ag_row[src[eidx]]
        sl_dstloc[slots] = (dl % 128).astype(np.float32)
        sl_edge[slots] = eidx
        sl_fill[slots] = True

        srcblk = sl_src_ag.reshape(NWIN, CW, 128).transpose(0, 2, 1)
        dlcblk = sl_dstloc.reshape(NWIN, CW, 128).transpose(0, 2, 1).astype(np.int32)
        meta = np.concatenate([srcblk, dlcblk], axis=2).astype(np.int32).copy()
        drow = to_bf16(sl_dstloc.reshape(NWIN, 1, CW * 128)).copy()

        gloc = np.full(NLOC, -1.0, np.float32)
        gloc[:nn] = (batch[ns:ne] - c * GPC).astype(np.float32)
        glocblk = to_bf16(gloc.reshape(NWIN, 128, 1)).copy()

        cores.append(dict(ns=ns, ne=ne, nn=nn, meta=meta, drow=drow,
                          glocblk=glocblk, sl_edge=sl_edge, sl_fill=sl_fill))
    return dict(cores=cores, gstart=gstart, NLOC=NLOC, NWIN=NWIN, CW=CW,
                GPC=GPC, n_cores=n_cores, gb=gb, NTAB=NTAB)


# ---------------- walrus sync-wait splitting ----------------
MAX_WAITS = 1

def split_waits(nc):
    eng_map = nc.engines
    for bbname, bassbb in nc.bb_map.items():
        insts = bassbb.bb.instructions
        i = 0
        while i < len(insts):
            inst = insts[i]
            si = inst.sync_info
            if si is not None and si.on_wait is not None and len(si.on_wait) > MAX_WAITS:
                waits = list(si.on_wait)
                si.on_wait = waits[-MAX_WAITS:]
                rest = waits[:-MAX_WAITS]
                for j in range(0, len(rest), MAX_WAITS):
                    eng = eng_map[inst.engine]
                    nop = eng.nop(nofuse=True)
                    nop_inst = nop.ins
                    for obb in nc.bb_map.values():
                        lst = obb.bb.instructions
                        for k in range(len(lst) - 1, -1, -1):
                            if lst[k].name == nop_inst.name:
                                del lst[k]
                                break
                    nsi = nop_inst.sync_info
                    chunk = rest[j:j + MAX_WAITS]
                    if nsi is None:
                        nop_inst.sync_info = mybir.SyncInfo(on_wait=chunk, on_update=[])
                    else:
                        nsi.on_wait = chunk
                    insts.insert(i, nop_inst)
                    i += 1
            i += 1


class TileContextFixed(tile.TileContext):
    def __exit__(self, *args):
        r = super().__exit__(*args)
        split_waits(self.nc)
        return r


# ---------------- weight pack layouts ----------------

def wpack_layout():
    """bf16 pack [128, WB]."""
    L = {}
    off = 0
    def add(name, cols):
        nonlocal off
        L[name] = (off, cols)
        off += cols
    add("lin1T", 256)          # rows 0:65
    add("W1T", 512)
    add("W2T", 256)            # rows 0:16
    add("attr_col", 2)
    for name in ("attl", "att0", "att1", "att2", "attm"):
        add(f"{name}_sq", 256)
    for i in range(4):
        add(f"WlTr{i}", 512)
        add(f"WrTr{i}", 512)
    for k in range(2):
        for b in range(2):
            add(f"gl2T_{k}{b}", 128)
    for g in range(5):
        for j in range(12):
            for b in range(2):
                add(f"gru{g}_w{j}{b}", 128)
    add("id", 128)
    add("bias_rows", 256 * 5)  # row0: ab0|ab1|ab2|gbias|molb
    add("w1T", 256)
    add("w2T", 64)
    add("w3T", 1)              # rows 0:65
    return L, off


def wpack32_layout():
    L = {}
    off = 0
    def add(name, cols):
        nonlocal off
        L[name] = (off, cols)
        off += cols
    for g in range(5):
        for b in range(2):
            add(f"gru{g}_brh{b}", 1)
            add(f"gru{g}_bzh{b}", 1)
            add(f"gru{g}_bin{b}", 1)
            add(f"gru{g}_bhn{b}", 1)
    add("mlp_b1", 1)
    add("mlp_b2", 1)
    for l in range(3):
        for b in range(2):
            add(f"ab{l}_{b}", 1)
    for b in range(2):
        add(f"gb_{b}", 1)
    for b in range(2):
        add(f"molb_{b}", 1)
    return L, off


def make_wpacks(inp):
    Lb, WB = wpack_layout()
    Lf, WF = wpack32_layout()
    Wb = np.zeros((128, WB), np.float32)
    Wf = np.zeros((128, WF), np.float32)
    f32 = lambda a: np.asarray(a, np.float32)

    def putb(name, arr, rows=128):
        o, c = Lb[name]
        assert arr.shape == (rows, c), (name, arr.shape, c)
        Wb[:rows, o:o + c] = arr
    def putf(name, arr, rows=128):
        o, c = Lf[name]
        assert arr.shape == (rows, c), (name, arr.shape)
        Wf[:rows, o:o + c] = arr

    putb("lin1T", np.concatenate([f32(inp['lin1_w']).T,
                                  f32(inp['lin1_b'])[None, :]], 0), rows=65)
    W1T = f32(inp['g_lin1_w'])[:, :256].T
    putb("W1T", np.concatenate([W1T[0:128], W1T[128:256]], 1))
    putb("W2T", f32(inp['g_lin1_w'])[:, 256:].T, rows=16)
    putb("attr_col", f32(inp['g_att_r']).reshape(2, 128).T)
    putb("attl_sq", np.tile(f32(inp['g_att_l']), (128, 1)))
    putb("attm_sq", np.tile(f32(inp['mol_att']), (128, 1)))
    for l in range(3):
        putb(f"att{l}_sq", np.tile(f32(inp['atom_att'][l]), (128, 1)))
    Wls = [inp['atom_Wl'][0], inp['atom_Wl'][1], inp['atom_Wl'][2], inp['mol_Wl']]
    Wrs = [inp['atom_Wr'][0], inp['atom_Wr'][1], inp['atom_Wr'][2], inp['mol_Wr']]
    for i in range(4):
        WT = f32(Wls[i]).T
        putb(f"WlTr{i}", np.concatenate([WT[0:128], WT[128:256]], 1))
        WT = f32(Wrs[i]).T
        putb(f"WrTr{i}", np.concatenate([WT[0:128], WT[128:256]], 1))
    g2T = f32(inp['g_lin2_w']).T
    for k in range(2):
        for b in range(2):
            putb(f"gl2T_{k}{b}", g2T[k * 128:(k + 1) * 128, b * 128:(b + 1) * 128])

    grus = [('gru0_wih', 'gru0_whh', 'gru0_bih', 'gru0_bhh', None),
            ('agru_wih', 'agru_whh', 'agru_bih', 'agru_bhh', 0),
            ('agru_wih', 'agru_whh', 'agru_bih', 'agru_bhh', 1),
            ('agru_wih', 'agru_whh', 'agru_bih', 'agru_bhh', 2),
            ('mgru_wih', 'mgru_whh', 'mgru_bih', 'mgru_bhh', None)]
    for g, (wi, wh, bi, bh, l) in enumerate(grus):
        wih = f32(inp[wi] if l is None else inp[wi][l])
        whh = f32(inp[wh] if l is None else inp[wh][l])
        bih = f32(inp[bi] if l is None else inp[bi][l])
        bhh = f32(inp[bh] if l is None else inp[bh][l])
        wihT = wih.T
        whhT = whh.T
        for k in range(2):
            for b in range(2):
                ks, bs = slice(k * 128, (k + 1) * 128), slice(b * 128, (b + 1) * 128)
                putb(f"gru{g}_w{0 + k}{b}", wihT[ks, 0:256][:, bs])
                putb(f"gru{g}_w{2 + k}{b}", whhT[ks, 0:256][:, bs])
                putb(f"gru{g}_w{4 + k}{b}", wihT[ks, 256:512][:, bs])
                putb(f"gru{g}_w{6 + k}{b}", whhT[ks, 256:512][:, bs])
                putb(f"gru{g}_w{8 + k}{b}", wihT[ks, 512:768][:, bs])
                putb(f"gru{g}_w{10 + k}{b}", whhT[ks, 512:768][:, bs])
        # gru input h' = elu+1 -> compensate with -rowsum(wih) on input biases
        rs_r = wih[0:256].sum(1)
        rs_z = wih[256:512].sum(1)
        rs_n = wih[512:768].sum(1)
        br = (bih[0:256] - rs_r + bhh[0:256]) * 0.5
        bz = (bih[256:512] - rs_z + bhh[256:512]) * 0.5
        bin_ = bih[512:768] - rs_n
        bhn = bhh[512:768]
        for b in range(2):
            bs = slice(b * 128, (b + 1) * 128)
            putf(f"gru{g}_brh{b}", br[bs].reshape(128, 1))
            putf(f"gru{g}_bzh{b}", bz[bs].reshape(128, 1))
            putf(f"gru{g}_bin{b}", bin_[bs].reshape(128, 1))
            putf(f"gru{g}_bhn{b}", bhn[bs].reshape(128, 1))

    putb("id", np.eye(128, dtype=np.float32))
    brow = np.zeros((1, 256 * 5), np.float32)
    for i in range(3):
        brow[0, i * 256:(i + 1) * 256] = f32(inp['atom_bias'][i])
    brow[0, 3 * 256:4 * 256] = f32(inp['g_bias'])
    brow[0, 4 * 256:5 * 256] = f32(inp['mol_bias'])
    putb("bias_rows", brow, rows=1)
    w1T = f32(inp['mlp_w1']).T
    putb("w1T", np.concatenate([w1T[0:128], w1T[128:256]], 1))
    putb("w2T", f32(inp['mlp_w2']).T)
    putb("w3T", np.concatenate([f32(inp['mlp_w3']).T,
                                f32(inp['mlp_b3']).reshape(1, 1)], 0), rows=65)
    putf("mlp_b1", f32(inp['mlp_b1']).reshape(128, 1))
    putf("mlp_b2", np.pad(f32(inp['mlp_b2']), (0, 64)).reshape(128, 1))
    for l in range(3):
        for b in range(2):
            putf(f"ab{l}_{b}", f32(inp['atom_bias'][l][b * 128:(b + 1) * 128]).reshape(128, 1))
    for b in range(2):
        putf(f"gb_{b}", f32(inp['g_bias'][b * 128:(b + 1) * 128]).reshape(128, 1))
    for b in range(2):
        putf(f"molb_{b}", f32(inp['mol_bias'][b * 128:(b + 1) * 128]).reshape(128, 1))
    return to_bf16(Wb).copy(), Wf


# ---------------- kernel builder ----------------

def build_kernel(NLOC, NWIN, CW, NG, n_cores, gb, NTAB, taps=()):
    H = 256
    EW = CW * 128
    assert NG in (128, 256)
    NGB = NG // 128
    nk = [gb[i + 1] - gb[i] for i in range(len(gb) - 1)]
    goff = np.concatenate([[0], np.cumsum([n_cores * k * 128 for k in nk])])

    nc = bass.Bass(num_devices=n_cores)
    Lb, WB = wpack_layout()
    Lf, WF = wpack32_layout()

    def dram_in(name, shape, dt=BF16):
        return nc.dram_tensor(name, list(shape), dt, kind="ExternalInput")

    xinT = dram_in("xinT", [65, NLOC])
    meta_d = dram_in("meta", [NWIN, 128, 2 * CW], I32)
    drow_d = dram_in("drow", [NWIN, 1, EW])
    gloc_d = dram_in("gloc", [NWIN, 128, 1])
    eaT_d = dram_in("eaT", [NWIN, 16, EW])
    wpack_d = dram_in("wpack", [128, WB])
    wpack32_d = dram_in("wpack32", [128, WF], F32)

    y = nc.dram_tensor("y", [1, NG], F32, kind="ExternalOutput")
    _tw = {(t if isinstance(t, str) else t[0]): (256 if isinstance(t, str) else t[1])
           for t in taps}
    dbg = {t: nc.dram_tensor(f"dbg_{t}", [NWIN, 128, w], BF16,
                             kind="ExternalOutput") for t, w in _tw.items()}

    cc_in = [nc.dram_tensor(f"cc_in{i}", [NLOC, H], BF16) for i in range(2)]
    tab = [nc.dram_tensor(f"tab{i}", [NTAB, H], BF16, addr_space="Shared")
           for i in range(2)]
    x_row = nc.dram_tensor("x_row", [NLOC, H], BF16)
    hl_md = nc.dram_tensor("hl_m", [NLOC, H], BF16)

    with TileContextFixed(nc) as tc, ExitStack() as ctx:
        wpool = ctx.enter_context(tc.tile_pool(name="weights", bufs=1))
        npool = ctx.enter_context(tc.tile_pool(name="node", bufs=2))
        cpool = ctx.enter_context(tc.tile_pool(name="chunk", bufs=2))
        spool = ctx.enter_context(tc.tile_pool(name="small", bufs=3))
        gpool = ctx.enter_context(tc.tile_pool(name="grup", bufs=1))
        mpool = ctx.enter_context(tc.tile_pool(name="meta", bufs=3))
        molpool = ctx.enter_context(tc.tile_pool(name="molp", bufs=1))
        pp_chps = ctx.enter_context(tc.tile_pool(name="pschps", bufs=1, space="PSUM"))
        pp_agg = ctx.enter_context(tc.tile_pool(name="psagg", bufs=2, space="PSUM"))
        pp_gru = ctx.enter_context(tc.tile_pool(name="psgru", bufs=2, space="PSUM"))
        pp_tab = ctx.enter_context(tc.tile_pool(name="pstab", bufs=1, space="PSUM"))

        wp = wpool.tile([128, WB], BF16, tag="wp")
        nc.sync.dma_start(wp[:], wpack_d.ap())
        wf = wpool.tile([128, WF], F32, tag="wf")
        nc.sync.dma_start(wf[:], wpack32_d.ap())

        def W(name):
            o, c = Lb[name]
            return wp[:, o:o + c]
        def Wrows(name, rows):
            o, c = Lb[name]
            return wp[0:rows, o:o + c]
        def Wf_(name):
            o, c = Lf[name]
            return wf[:, o:o + c]
        def bias_row(idx, b):
            o, c = Lb["bias_rows"]
            return wp[0:1, o + idx * 256 + b * 128: o + idx * 256 + (b + 1) * 128]

        xT_t = [None] * NWIN
        hr_t = [None] * NWIN
        wc_all = wpool.tile([128, NWIN], BF16, tag="wc_all")
        ones_row = wpool.tile([1, 128], BF16, tag="ones_row")
        nc.vector.memset(ones_row[:], 1.0)
        iota_col_b = wpool.tile([128, 1], BF16, tag="iotacolb")
        iota_i = wpool.tile([128, 1], I32, tag="iotacoli")
        nc.gpsimd.iota(iota_i[:], pattern=[[0, 1]], base=0, channel_multiplier=1)
        nc.vector.tensor_copy(iota_col_b[:], iota_i[:])
        iota_row_i = wpool.tile([128, 128], I32, tag="iotarow")
        nc.gpsimd.iota(iota_row_i[:], pattern=[[1, 128]], base=0, channel_multiplier=0)
        iotaNG_b = wpool.tile([128, NG], BF16, tag="iotaNG")
        iotaNG_i = spool.tile([128, NG], I32, tag="iotaNGi", bufs=1)
        nc.gpsimd.iota(iotaNG_i[:], pattern=[[1, NG]], base=0, channel_multiplier=0)
        nc.vector.tensor_copy(iotaNG_b[:], iotaNG_i[:])

        def lrelu_dve(out_ap, in_ap):
            nc.vector.scalar_tensor_tensor(out=out_ap, in0=in_ap, scalar=0.01,
                                           in1=in_ap, op0=alu.mult, op1=alu.max)

        def tap(name, w, tile_ap):
            if name in dbg:
                rows = tile_ap.shape[0]
                nc.sync.dma_start(dbg[name].ap()[w][0:rows, :], tile_ap)

        def collective(gi, parity):
            r0, r1 = gb[gi] * 128, gb[gi + 1] * 128
            o0 = int(goff[gi])
            rows = n_cores * (r1 - r0)
            if n_cores == 1:
                nc.sync.dma_start(tab[parity].ap()[o0:o0 + rows, :],
                                  cc_in[parity].ap()[r0:r1, :])
            else:
                nc.gpsimd.collective_compute(
                    "AllGather", alu.bypass,
                    replica_groups=[list(range(n_cores))],
                    ins=[cc_in[parity].ap()[r0:r1, :]],
                    outs=[tab[parity].ap()[o0:o0 + rows, :]])

        # ---------------- GRU (transposed, tanh-only, elu+1-compensated) ---
        def gru(g, hT_ap, xT_ap, xn_out, WIDE):
            rps = pp_gru.tile([128, 2 * WIDE], F32, tag="gp", name="rps")
            zps = pp_gru.tile([128, 2 * WIDE], F32, tag="gp", name="zps")
            for b in range(2):
                bs = slice(b * WIDE, (b + 1) * WIDE)
                for k in range(2):
                    ks = slice(k * WIDE, (k + 1) * WIDE)
                    nc.tensor.matmul(rps[:, bs], lhsT=W(f"gru{g}_w{0 + k}{b}"),
                                     rhs=hT_ap[:, ks], start=(k == 0), stop=False)
                for k in range(2):
                    ks = slice(k * WIDE, (k + 1) * WIDE)
                    nc.tensor.matmul(rps[:, bs], lhsT=W(f"gru{g}_w{2 + k}{b}"),
                                     rhs=xT_ap[:, ks], start=False, stop=(k == 1))
                for k in range(2):
                    ks = slice(k * WIDE, (k + 1) * WIDE)
                    nc.tensor.matmul(zps[:, bs], lhsT=W(f"gru{g}_w{4 + k}{b}"),
                                     rhs=hT_ap[:, ks], start=(k == 0), stop=False)
                for k in range(2):
                    ks = slice(k * WIDE, (k + 1) * WIDE)
                    nc.tensor.matmul(zps[:, bs], lhsT=W(f"gru{g}_w{6 + k}{b}"),
                                     rhs=xT_ap[:, ks], start=False, stop=(k == 1))
            r_t = gpool.tile([128, 2 * WIDE], BF16, tag=f"gru_r{WIDE}", name="grur")
            z_t = gpool.tile([128, 2 * WIDE], BF16, tag=f"gru_z{WIDE}", name="gruz")
            for b in range(2):
                bs = slice(b * WIDE, (b + 1) * WIDE)
                nc.scalar.activation(r_t[:, bs], rps[:, bs], act.Tanh,
                                     bias=Wf_(f"gru{g}_brh{b}"), scale=0.5)
                nc.scalar.activation(z_t[:, bs], zps[:, bs], act.Tanh,
                                     bias=Wf_(f"gru{g}_bzh{b}"), scale=0.5)
            ips = pp_gru.tile([128, 2 * WIDE], F32, tag="gp", name="ips")
            hps = pp_gru.tile([128, 2 * WIDE], F32, tag="gp", name="hps")
            for b in range(2):
                bs = slice(b * WIDE, (b + 1) * WIDE)
                for k in range(2):
                    ks = slice(k * WIDE, (k + 1) * WIDE)
                    nc.tensor.matmul(ips[:, bs], lhsT=W(f"gru{g}_w{8 + k}{b}"),
                                     rhs=hT_ap[:, ks], start=(k == 0), stop=(k == 1))
                for k in range(2):
                    ks = slice(k * WIDE, (k + 1) * WIDE)
                    nc.tensor.matmul(hps[:, bs], lhsT=W(f"gru{g}_w{10 + k}{b}"),
                                     rhs=xT_ap[:, ks], start=(k == 0), stop=(k == 1))
            # n = tanh(ips + bin + 0.5*(v + v*r_t)), v = hps + bhn
            v = gpool.tile([128, 2 * WIDE], BF16, tag=f"gru_v{WIDE}", name="gruv")
            for b in range(2):
                bs = slice(b * WIDE, (b + 1) * WIDE)
                nc.vector.tensor_scalar(out=v[:, bs], in0=hps[:, bs],
                                        scalar1=Wf_(f"gru{g}_bhn{b}"), scalar2=None,
                                        op0=alu.add)
            w_ = gpool.tile([128, 2 * WIDE], BF16, tag=f"gru_w{WIDE}", name="gruw")
            nc.vector.tensor_tensor(out=w_[:], in0=v[:], in1=r_t[:], op=alu.mult)
            s1 = gpool.tile([128, 2 * WIDE], BF16, tag=f"gru_s1{WIDE}", name="grus1")
            nc.vector.tensor_tensor(out=s1[:], in0=v[:], in1=w_[:], op=alu.add)
            t2 = gpool.tile([128, 2 * WIDE], BF16, tag=f"gru_t2{WIDE}", name="grut2")
            nc.vector.scalar_tensor_tensor(out=t2[:], in0=s1[:], scalar=0.5,
                                           in1=ips[:], op0=alu.mult, op1=alu.add)
            n_t = gpool.tile([128, 2 * WIDE], BF16, tag=f"gru_n{WIDE}", name="grun")
            for b in range(2):
                bs = slice(b * WIDE, (b + 1) * WIDE)
                nc.scalar.activation(n_t[:, bs], t2[:, bs], act.Tanh,
                                     bias=Wf_(f"gru{g}_bin{b}"))
            # x' = relu(0.5*(n + x + z_t*(x - n)))   (z-path on gpsimd)
            a_ = gpool.tile([128, 2 * WIDE], BF16, tag=f"gru_a{WIDE}", name="grua")
            nc.vector.tensor_tensor(out=a_[:], in0=xT_ap, in1=n_t[:], op=alu.subtract)
            m_ = gpool.tile([128, 2 * WIDE], BF16, tag=f"gru_m{WIDE}", name="grum")
            nc.vector.tensor_tensor(out=m_[:], in0=z_t[:], in1=a_[:], op=alu.mult)
            s2 = gpool.tile([128, 2 * WIDE], BF16, tag=f"gru_s2{WIDE}", name="grus2")
            nc.vector.tensor_tensor(out=s2[:], in0=n_t[:], in1=m_[:], op=alu.add)
            s3 = gpool.tile([128, 2 * WIDE], BF16, tag=f"gru_s3{WIDE}", name="grus3")
            nc.vector.tensor_tensor(out=s3[:], in0=s2[:], in1=xT_ap, op=alu.add)
            nc.vector.tensor_scalar(out=xn_out, in0=s3[:], scalar1=0.5, scalar2=0.0,
                                    op0=alu.mult, op1=alu.max)

        # ================= P0: input projection =================
        for w in range(NWIN):
            sl = slice(w * 128, (w + 1) * 128)
            xin_t = mpool.tile([65, 128], BF16, tag="xin")
            nc.sync.dma_start(xin_t[:], xinT.ap()[:, sl])
            x0ps = pp_tab.tile([128, 256], F32, tag="tabps", name="x0ps")
            for b in range(2):
                nc.tensor.matmul(x0ps[:, b * 128:(b + 1) * 128],
                                 lhsT=Wrows("lin1T", 65)[:, b * 128:(b + 1) * 128],
                                 rhs=xin_t[:], start=True, stop=True)
            xt = npool.tile([128, 256], BF16, tag=f"xT_{w}", bufs=1)
            nc.scalar.activation(xt[:], x0ps[:], act.Lrelu, alpha=0.01)
            xT_t[w] = xt
            tap('x0', w, xt[:])
            ups = pp_agg.tile([128, 512], F32, tag="aggps", name="ups")
            for b in range(2):
                nc.tensor.matmul(ups[:, 0:256], lhsT=xt[:, b * 128:(b + 1) * 128],
                                 rhs=W("W1T")[:, b * 256:(b + 1) * 256],
                                 start=(b == 0), stop=(b == 1))
            for b in range(2):
                nc.tensor.matmul(ups[:, 256:257], lhsT=xt[:, b * 128:(b + 1) * 128],
                                 rhs=W("attr_col")[:, b:b + 1],
                                 start=(b == 0), stop=(b == 1))
            u_sb = npool.tile([128, 256], BF16, tag="tabsb", name="usb")
            nc.vector.tensor_copy(u_sb[:], ups[:, 0:256])
            nc.vector.tensor_copy(wc_all[:, w:w + 1], ups[:, 256:257])
            nc.sync.dma_start(cc_in[0].ap()[sl, :], u_sb[:])
            for gi in range(len(nk)):
                if w == gb[gi + 1] - 1:
                    collective(gi, 0)

        # ================= edge layers =================
        def edge_layer(kind, l, parity, last=False):
            gru_i = 0 if kind == 'gate' else 1 + l
            attw = W("attl_sq") if kind == 'gate' else W(f"att{l}_sq")
            tab_rd = tab[parity]
            nparity = 1 - parity
            for w in range(NWIN):
                sl = slice(w * 128, (w + 1) * 128)
                meta_t = mpool.tile([128, 2 * CW], I32, tag="meta")
                nc.sync.dma_start(meta_t[:], meta_d.ap()[w])
                drow_t = mpool.tile([1, EW], BF16, tag="drow")
                nc.sync.dma_start(drow_t[:], drow_d.ap()[w])
                g = cpool.tile([128, CW * H], BF16, tag="gather", bufs=2)
                for ci in range(CW):
                    nc.gpsimd.indirect_dma_start(
                        out=g[:, ci * H:(ci + 1) * H], out_offset=None,
                        in_=tab_rd.ap(),
                        in_offset=IndirectOffsetOnAxis(ap=meta_t[:, ci:ci + 1],
                                                       axis=0))
                sel0 = cpool.tile([128, EW], BF16, tag="sel0")
                nc.vector.tensor_tensor(
                    out=sel0[:].rearrange('p (c n) -> p c n', c=CW),
                    in0=iota_row_i[:].unsqueeze(1).broadcast_to([128, CW, 128]),
                    in1=meta_t[:, CW:2 * CW].to_broadcast([128, CW, 128]),
                    op=alu.is_equal)
                chps = pp_chps.tile([128, 1536], F32, tag="chps", name="chps")
                # drow broadcast via PE into chps[:, 0:EW], then selT = (drow==d)
                nc.tensor.matmul(chps[:, 0:512], lhsT=ones_row[:],
                                 rhs=drow_t[:, 0:512], start=True, stop=True)
                nc.tensor.matmul(chps[:, 512:EW], lhsT=ones_row[:],
                                 rhs=drow_t[:, 512:EW], start=True, stop=True)
                selT = cpool.tile([128, EW], BF16, tag="selT")
                nc.vector.tensor_tensor(out=selT[:], in0=chps[:, 0:EW],
                                        in1=iota_col_b[:].to_broadcast([128, EW]),
                                        op=alu.is_equal)
                if kind == 'atom':
                    hrw = hr_t[w]
                    for ci in range(CW):
                        nc.tensor.matmul(chps[:, ci * 256:(ci + 1) * 256],
                                         lhsT=selT[:, ci * 128:(ci + 1) * 128],
                                         rhs=hrw[:], start=True, stop=True)
                else:
                    eat = mpool.tile([16, EW], BF16, tag="eat", bufs=2)
                    nc.sync.dma_start(eat[:], eaT_d.ap()[w])
                    for ci in range(CW):
                        nc.tensor.matmul(chps[:, ci * 256:(ci + 1) * 256],
                                         lhsT=eat[:, ci * 128:(ci + 1) * 128],
                                         rhs=Wrows("W2T", 16), start=True, stop=True)
                tap(f'g_{kind}{l}', w, g[:])
                t_t = cpool.tile([128, CW * H], BF16, tag="t_t")
                nc.vector.scalar_tensor_tensor(out=t_t[:], in0=chps[:, 0:CW * H],
                                               scalar=0.0, in1=g[:],
                                               op0=alu.add, op1=alu.add)
                tl = cpool.tile([128, CW * H], BF16, tag="tl")
                lrelu_dve(tl[:], t_t[:])
                escr = cpool.tile([128, CW * H], BF16, tag="t_t")
                nc.vector.tensor_tensor(
                    out=escr[:].rearrange('p (c n) -> p c n', c=CW),
                    in0=tl[:].rearrange('p (c n) -> p c n', c=CW),
                    in1=attw.unsqueeze(1).broadcast_to([128, CW, 256]),
                    op=alu.mult)
                ecol = spool.tile([128, CW], F32, tag="ecol")
                nc.vector.tensor_reduce(out=ecol[:],
                                        in_=escr[:].rearrange('p (c n) -> p c n', c=CW),
                                        axis=mybir.AxisListType.X, op=alu.add)
                agg = pp_agg.tile([128, 512], F32, tag="aggps", name="agg")
                ex = spool.tile([128, CW], BF16, tag="ex")
                if kind == 'gate':
                    for ci in range(CW):
                        nc.tensor.matmul(agg[:, 384 + ci:385 + ci],
                                         lhsT=selT[:, ci * 128:(ci + 1) * 128],
                                         rhs=wc_all[:, w:w + 1],
                                         start=(ci == 0) if CW == 1 else True,
                                         stop=True)
                    e2 = spool.tile([128, CW], F32, tag="e2")
                    nc.vector.scalar_tensor_tensor(out=e2[:], in0=agg[:, 384:384 + CW],
                                                   scalar=0.0, in1=ecol[:],
                                                   op0=alu.add, op1=alu.add)
                    el = spool.tile([128, CW], F32, tag="el")
                    lrelu_dve(el[:], e2[:])
                    nc.scalar.activation(ex[:], el[:], act.Exp)
                else:
                    nc.scalar.activation(ex[:], ecol[:], act.Exp)
                tap(f'tl_{kind}{l}', w, tl[:])
                tap(f'ex_{kind}{l}', w, ex[:])
                val = cpool.tile([128, CW * H], BF16, tag="tl")
                vsrc = g if kind == 'atom' else tl
                nc.vector.tensor_tensor(
                    out=val[:].rearrange('p (c n) -> p c n', c=CW),
                    in0=vsrc[:].rearrange('p (c n) -> p c n', c=CW),
                    in1=ex[:].to_broadcast([128, CW, 256]), op=alu.mult)
                tap(f'val_{kind}{l}', w, val[:])
                for b in range(2):
                    for ci in range(CW):
                        nc.tensor.matmul(
                            agg[:, b * 128:(b + 1) * 128],
                            lhsT=val[:, ci * 256 + b * 128: ci * 256 + (b + 1) * 128],
                            rhs=sel0[:, ci * 128:(ci + 1) * 128],
                            start=(ci == 0), stop=(ci == CW - 1))
                for ci in range(CW):
                    nc.tensor.matmul(agg[0:1, 256:384], lhsT=ex[:, ci:ci + 1],
                                     rhs=sel0[:, ci * 128:(ci + 1) * 128],
                                     start=(ci == 0), stop=(ci == CW - 1))
                srow = spool.tile([1, 128], F32, tag="srow")
                nc.vector.tensor_scalar(out=srow[:], in0=agg[0:1, 256:384],
                                        scalar1=EPS, scalar2=None, op0=alu.max)
                srow_b = spool.tile([1, 128], BF16, tag="srowb")
                nc.vector.tensor_copy(srow_b[:], srow[:])
                tap(f'srowb_{kind}{l}', w, srow_b[:])
                rrow = spool.tile([1, 128], F32, tag="rrow")
                nc.vector.reciprocal(rrow[:], srow[:])
                rrow_b = spool.tile([1, 128], BF16, tag="rrowb")
                nc.vector.tensor_copy(rrow_b[:], rrow[:])
                rbps = pp_tab.tile([128, 256], F32, tag="tabps", name="rbps")
                nc.tensor.matmul(rbps[:, 0:128], lhsT=ones_row[:], rhs=rrow_b[:],
                                 start=True, stop=True)
                rbc = npool.tile([128, 128], F32, tag="rbc")
                nc.vector.tensor_copy(rbc[:], rbps[:, 0:128])
                if kind == 'atom':
                    qsrc = agg[:, 0:256]
                else:
                    A_sb = npool.tile([128, 256], BF16, tag="A_sb")
                    nc.vector.tensor_copy(A_sb[:], agg[:, 0:256])
                    tap(f'Asb_{kind}{l}', w, A_sb[:])
                    g2ps = pp_gru.tile([128, 256], F32, tag="gp", name="g2ps")
                    for b in range(2):
                        for k in range(2):
                            nc.tensor.matmul(g2ps[:, b * 128:(b + 1) * 128],
                                             lhsT=W(f"gl2T_{k}{b}"),
                                             rhs=A_sb[:, k * 128:(k + 1) * 128],
                                             start=(k == 0), stop=(k == 1))
                    qsrc = g2ps[:, 0:256]
                q = npool.tile([128, 256], BF16, tag="q")
                nc.vector.scalar_tensor_tensor(
                    out=q[:].rearrange('p (b n) -> p b n', b=2),
                    in0=qsrc.rearrange('p (b n) -> p b n', b=2),
                    scalar=0.0,
                    in1=rbc[:].unsqueeze(1).broadcast_to([128, 2, 128]),
                    op0=alu.add, op1=alu.mult)
                tap(f'q_{kind}{l}', w, q[:])
                bkeys = ([f"ab{l}_0", f"ab{l}_1"] if kind == 'atom'
                         else ["gb_0", "gb_1"])
                u_t = npool.tile([128, 256], BF16, tag="u_t")
                p_t = npool.tile([128, 256], BF16, tag="p_t")
                for b in range(2):
                    bs = slice(b * 128, (b + 1) * 128)
                    nc.vector.tensor_scalar(out=u_t[:, bs], in0=q[:, bs],
                                            scalar1=Wf_(bkeys[b]), scalar2=0.0,
                                            op0=alu.add, op1=alu.min)
                    nc.vector.tensor_scalar(out=p_t[:, bs], in0=q[:, bs],
                                            scalar1=Wf_(bkeys[b]), scalar2=0.0,
                                            op0=alu.add, op1=alu.max)
                eu = npool.tile([128, 256], BF16, tag="eu")
                nc.scalar.activation(eu[:], u_t[:], act.Exp)
                hT = npool.tile([128, 256], BF16, tag="hT")
                nc.vector.tensor_tensor(out=hT[:], in0=p_t[:], in1=eu[:], op=alu.add)
                tap(f'hT_{kind}{l}', w, hT[:])
                xt_new = npool.tile([128, 256], BF16, tag=f"xT_{w}", bufs=1)
                gru(gru_i, hT[:], xT_t[w][:], xt_new[:], 128)
                xT_t[w] = xt_new
                tap(f'x_{kind}{l}', w, xt_new[:])
                if not last:
                    nl = (l + 1) if kind == 'atom' else 0
                    hlps = pp_tab.tile([128, 256], F32, tag="tabps", name="hlps")
                    for k in range(2):
                        nc.tensor.matmul(hlps[:], lhsT=xt_new[:, k * 128:(k + 1) * 128],
                                         rhs=W(f"WlTr{nl}")[:, k * 256:(k + 1) * 256],
                                         start=(k == 0), stop=(k == 1))
                    hlsb = npool.tile([128, 256], BF16, tag="tabsb", name="hlsb")
                    nc.vector.tensor_copy(hlsb[:], hlps[:])
                    nc.sync.dma_start(cc_in[nparity].ap()[sl, :], hlsb[:])
                    hrps = pp_tab.tile([128, 256], F32, tag="tabps", name="hrps")
                    for k in range(2):
                        nc.tensor.matmul(hrps[:], lhsT=xt_new[:, k * 128:(k + 1) * 128],
                                         rhs=W(f"WrTr{nl}")[:, k * 256:(k + 1) * 256],
                                         start=(k == 0), stop=(k == 1))
                    hrsb = npool.tile([128, 256], BF16, tag=f"hr_{w}", bufs=1)
                    nc.vector.tensor_copy(hrsb[:], hrps[:])
                    hr_t[w] = hrsb
                    for gi in range(len(nk)):
                        if w == gb[gi + 1] - 1:
                            collective(gi, nparity)
                else:
                    xrps = pp_tab.tile([128, 256], F32, tag="tabps", name="xrps")
                    for k in range(2):
                        nc.tensor.matmul(xrps[:, k * 128:(k + 1) * 128],
                                         lhsT=xt_new[:, k * 128:(k + 1) * 128],
                                         rhs=W("id"), start=True, stop=True)
                    xrsb = npool.tile([128, 256], BF16, tag="tabsb", name="xrsb")
                    nc.vector.tensor_copy(xrsb[:], xrps[:])
                    nc.sync.dma_start(x_row.ap()[sl, :], xrsb[:])
                    hmps = pp_tab.tile([128, 256], F32, tag="tabps", name="hmps")
                    for k in range(2):
                        nc.tensor.matmul(hmps[:], lhsT=xt_new[:, k * 128:(k + 1) * 128],
                                         rhs=W("WlTr3")[:, k * 256:(k + 1) * 256],
                                         start=(k == 0), stop=(k == 1))
                    hmsb = npool.tile([128, 256], BF16, tag="tabsb", name="hmsb")
                    nc.vector.tensor_copy(hmsb[:], hmps[:])
                    nc.sync.dma_start(hl_md.ap()[sl, :], hmsb[:])

        edge_layer('gate', 0, 0)
        edge_layer('atom', 0, 1)
        edge_layer('atom', 1, 0)
        edge_layer('atom', 2, 1, last=True)

        # ================= mol phase =================
        glc_cache = []
        for w in range(NWIN):
            t = wpool.tile([128, 1], BF16, tag=f"glcc_{w}", name="glcc")
            nc.sync.dma_start(t[:], gloc_d.ap()[w])
            glc_cache.append(t)

        def make_selg(w):
            selg = spool.tile([128, NG], BF16, tag="selg")
            nc.vector.tensor_tensor(out=selg[:], in0=iotaNG_b[:],
                                    in1=glc_cache[w][:].to_broadcast([128, NG]),
                                    op=alu.is_equal)
            return selg

        ro_ps = pp_chps.tile([128, 1536], F32, tag="chps", name="rops")
        for w in range(NWIN):
            selg = make_selg(w)
            xr = cpool.tile([128, H], BF16, tag="xr")
            nc.sync.dma_start(xr[:], x_row.ap()[w * 128:(w + 1) * 128, :])
            for b in range(2):
                nc.tensor.matmul(ro_ps[:, b * 512:b * 512 + NG],
                                 lhsT=xr[:, b * 128:(b + 1) * 128], rhs=selg[:],
                                 start=(w == 0), stop=(w == NWIN - 1))
        outT = molpool.tile([128, 2 * NG], BF16, tag="outT0")
        for b in range(2):
            nc.scalar.activation(outT[:, b * NG:(b + 1) * NG],
                                 ro_ps[:, b * 512:b * 512 + NG], act.Relu)

        for step in range(3):
            hrm = molpool.tile([128, NGB * 256], BF16, tag="hrm")
            for gb_ in range(NGB):
                hrps = pp_tab.tile([128, 256], F32, tag="tabps", name="hrpsm")
                for k in range(2):
                    nc.tensor.matmul(
                        hrps[:],
                        lhsT=outT[:, k * NG + gb_ * 128: k * NG + gb_ * 128 + 128],
                        rhs=W("WrTr3")[:, k * 256:(k + 1) * 256],
                        start=(k == 0), stop=(k == 1))
                nc.vector.tensor_copy(hrm[:, gb_ * 256:(gb_ + 1) * 256], hrps[:])

            agm = pp_agg.tile([128, 512], F32, tag="aggps", name="agm")
            sgm = pp_agg.tile([128, 512], F32, tag="aggps", name="sgm")
            for w in range(NWIN):
                selg = make_selg(w)
                Bps = pp_gru.tile([128, 256], F32, tag="gp", name="Bps")
                for gb_ in range(NGB):
                    tps = pp_tab.tile([128, 256], BF16, tag="tabps", name="tpsm")
                    nc.tensor.transpose(out=tps[:, 0:128],
                                        in_=selg[:, gb_ * 128:(gb_ + 1) * 128],
                                        identity=W("id"))
                    sTg = spool.tile([128, 128], BF16, tag="sTg")
                    nc.vector.tensor_copy(sTg[:], tps[:, 0:128])
                    nc.tensor.matmul(Bps[:, 0:256], lhsT=sTg[:],
                                     rhs=hrm[:, gb_ * 256:(gb_ + 1) * 256],
                                     start=(gb_ == 0), stop=(gb_ == NGB - 1))
                hmw = cpool.tile([128, H], BF16, tag="hmw")
                nc.sync.dma_start(hmw[:], hl_md.ap()[w * 128:(w + 1) * 128, :])
                ttm = cpool.tile([128, 256], BF16, tag="ttm")
                nc.vector.scalar_tensor_tensor(out=ttm[:], in0=Bps[:, 0:256],
                                               scalar=0.0, in1=hmw[:],
                                               op0=alu.add, op1=alu.add)
                tlm = cpool.tile([128, 256], BF16, tag="tlm")
                lrelu_dve(tlm[:], ttm[:])
                escr_m = cpool.tile([128, 256], BF16, tag="escrm")
                nc.vector.tensor_tensor(out=escr_m[:], in0=tlm[:], in1=W("attm_sq"),
                                        op=alu.mult)
                ecol_m = spool.tile([128, 1], F32, tag="ecolm")
                nc.vector.tensor_reduce(out=ecol_m[:], in_=escr_m[:],
                                        axis=mybir.AxisListType.X, op=alu.add)
                exm = spool.tile([128, 1], BF16, tag="exm")
                nc.scalar.activation(exm[:], ecol_m[:], act.Exp)
                valm = cpool.tile([128, 256], BF16, tag="valm")
                nc.vector.tensor_tensor(out=valm[:], in0=hmw[:],
                                        in1=exm[:].to_broadcast([128, 256]),
                                        op=alu.mult)
                for b in range(2):
                    nc.tensor.matmul(agm[:, b * NG:(b + 1) * NG],
                                     lhsT=valm[:, b * 128:(b + 1) * 128],
                                     rhs=selg[:], start=(w == 0),
                                     stop=(w == NWIN - 1))
                nc.tensor.matmul(sgm[0:1, 0:NG], lhsT=exm[:], rhs=selg[:],
                                 start=(w == 0), stop=(w == NWIN - 1))
            srowm = spool.tile([1, NG], F32, tag="srowm")
            nc.vector.tensor_scalar(out=srowm[:], in0=sgm[0:1, 0:NG],
                                    scalar1=EPS, scalar2=None, op0=alu.max)
            srowm_b = spool.tile([1, NG], BF16, tag="srowmb")
            nc.vector.tensor_copy(srowm_b[:], srowm[:])
            rrowm = spool.tile([1, NG], F32, tag="rrowm")
            nc.vector.reciprocal(rrowm[:], srowm[:])
            rrowm_b = spool.tile([1, NG], BF16, tag="rrowmb")
            nc.vector.tensor_copy(rrowm_b[:], rrowm[:])
            rbmps = pp_tab.tile([128, 256], F32, tag="tabps", name="rbmps")
            nc.tensor.matmul(rbmps[:, 0:NG], lhsT=ones_row[:], rhs=rrowm_b[:],
                             start=True, stop=True)
            rbcm = npool.tile([128, NG], F32, tag="rbcm", bufs=1)
            nc.vector.tensor_copy(rbcm[:], rbmps[:, 0:NG])
            qm = molpool.tile([128, 2 * NG], BF16, tag="qm")
            nc.vector.scalar_tensor_tensor(
                out=qm[:].rearrange('p (b n) -> p b n', b=2),
                in0=agm[:, 0:2 * NG].rearrange('p (b n) -> p b n', b=2),
                scalar=0.0,
                in1=rbcm[:].unsqueeze(1).broadcast_to([128, 2, NG]),
                op0=alu.add, op1=alu.mult)
            um = molpool.tile([128, 2 * NG], BF16, tag="um")
            pm = molpool.tile([128, 2 * NG], BF16, tag="pm")
            for b in range(2):
                bs = slice(b * NG, (b + 1) * NG)
                nc.vector.tensor_scalar(out=um[:, bs], in0=qm[:, bs],
                                        scalar1=Wf_(f"molb_{b}"), scalar2=0.0,
                                        op0=alu.add, op1=alu.min)
                nc.vector.tensor_scalar(out=pm[:, bs], in0=qm[:, bs],
                                        scalar1=Wf_(f"molb_{b}"), scalar2=0.0,
                                        op0=alu.add, op1=alu.max)
            eum = molpool.tile([128, 2 * NG], BF16, tag="eum")
            nc.scalar.activation(eum[:], um[:], act.Exp)
            hTm = molpool.tile([128, 2 * NG], BF16, tag="qm")
            nc.vector.tensor_tensor(out=hTm[:], in0=pm[:], in1=eum[:], op=alu.add)
            out_new = molpool.tile([128, 2 * NG], BF16, tag=f"outT{(step + 1) % 2}")
            gru(4, hTm[:], outT[:], out_new[:], NG)
            outT = out_new

        # ================= MLP head =================
        o1ps = pp_tab.tile([128, 256], F32, tag="tabps", name="o1ps")
        for k in range(2):
            nc.tensor.matmul(o1ps[:, 0:NG], lhsT=W("w1T")[:, k * 128:(k + 1) * 128],
                             rhs=outT[:, k * NG:(k + 1) * NG],
                             start=(k == 0), stop=(k == 1))
        o1 = npool.tile([128, NG], BF16, tag="o1")
        nc.scalar.activation(o1[:], o1ps[:, 0:NG], act.Relu, bias=Wf_("mlp_b1"))
        o2ps = pp_gru.tile([64, 256], F32, tag="gp", name="o2ps")
        nc.tensor.matmul(o2ps[:, 0:NG], lhsT=W("w2T"), rhs=o1[:], start=True, stop=True)
        o2 = npool.tile([65, NG], BF16, tag="o2")
        nc.vector.memset(o2[64:65, :], 1.0)
        nc.scalar.activation(o2[0:64, :], o2ps[0:64, 0:NG], act.Relu,
                             bias=Wf_("mlp_b2")[0:64, :])
        o3ps = pp_tab.tile([1, 256], F32, tag="tabps", name="o3ps")
        nc.tensor.matmul(o3ps[0:1, 0:NG], lhsT=Wrows("w3T", 65), rhs=o2[:],
                         start=True, stop=True)
        o3 = spool.tile([1, NG], F32, tag="o3")
        nc.vector.tensor_copy(o3[:], o3ps[0:1, 0:NG])
        nc.sync.dma_start(y.ap()[:, 0:NG], o3[:])

    return nc


# ---------------- host-side per-core inputs ----------------

def make_core_inputs(P, inputs, ci, wb, wf):
    c = P['cores'][ci]
    NLOC, NWIN, CW = P['NLOC'], P['NWIN'], P['CW']
    x = np.asarray(inputs['x'], np.float32)
    xinT = np.zeros((65, NLOC), np.float32)
    xinT[:64, :c['nn']] = x[c['ns']:c['ne']].T
    xinT[64, :] = 1.0
    ea = np.asarray(inputs['edge_attr'], np.float32)
    ea_perm = np.where(c['sl_fill'][:, None], ea[c['sl_edge']], 0.0).astype(np.float32)
    NCH = NWIN * CW
    eaT = ea_perm.reshape(NWIN, CW * 128, 16).transpose(0, 2, 1).copy()
    return dict(
        xinT=to_bf16(xinT).copy(),
        meta=c['meta'], drow=c['drow'], gloc=c['glocblk'],
        eaT=to_bf16(eaT).copy(),
        wpack=wb, wpack32=wf,
    )


_CACHE = {}
LAST_EXEC_NS = None


def kernel(**inputs):
    inputs = dict(inputs)
    edge_index = np.asarray(inputs['edge_index']).astype(np.int64)
    batch = np.asarray(inputs['batch']).astype(np.int64)
    n_cores = 8
    G = 2048
    P = preprocess(edge_index, batch, n_cores=n_cores, G=G, CW=5)
    key = (P['NLOC'], P['NWIN'], P['CW'], P['GPC'], P['gb'], P['NTAB'])
    if key not in _CACHE:
        _CACHE[key] = build_kernel(P['NLOC'], P['NWIN'], P['CW'], P['GPC'],
                                   n_cores, P['gb'], P['NTAB'])
    nc = _CACHE[key]
    wb, wf = make_wpacks(inputs)
    ins = [make_core_inputs(P, inputs, ci, wb, wf) for ci in range(n_cores)]
    from concourse.bass_utils import run_bass_kernel_spmd
    trace = bool(os.environ.get('BASS_KERNEL_TRACE'))
    res = run_bass_kernel_spmd(nc, ins, list(range(n_cores)), trace=trace)
    if trace:
        global LAST_EXEC_NS
        LAST_EXEC_NS = res.exec_time_ns
    y = np.concatenate([res.results[c]['y'][0, :P['GPC']] for c in range(n_cores)])
    return y.reshape(G, 1).astype(np.float32)
